# revision 8
# baseline (speedup 1.0000x reference)
"""2-layer multi-edge-type GAT on Trainium2, 8-core SPMD (v2).

Strategy: shard edges by dst-owning core. Host bin-packs each core's dst
nodes into 32-slot windows jointly balanced over both edge types so every
window holds <=512 edges per type (exactly 4 chunks of 128) -- no per-window
chunk-count variation, ~5% padding. Per (core, type) the src node set is
compacted (<32767 entries) so feat rows can be fetched with batched int16
`dma_gather` instructions (one per 64 chunks instead of one indirect DMA per
chunk).

Device per core:
  A) er0 table: er = (feat @ W0)*attn_r for the core's permuted dst list,
     written as 256B rows for batched gathers; a dump row of -1000 makes
     padding edges vanish (exp(lrelu(el-1000)) == 0).
  B) Layer-0 edges (2 types): per group of 64 chunks, one transposed
     dma_gather of fp16 feat columns + one dma_gather of er rows; per chunk
     a fp16 matmul feat.T @ [W0|W0*attn_l] gives [z|el] in PSUM; DVE builds
     ex=exp(leakyrelu(el+er)) and rows=[ex*z|ex] fp16; one fp16 one-hot
     window matmul accumulates [num|den] per 32-dst window in PSUM; window
     flush normalizes (softmax shift dropped -- logits O(1), fp32-safe).
  C) h = relu(mean_h(gat_a+gat_b)); z1el rows computed shard-wise,
     AllGathered (compact 34 cols) then restrided to 256B rows.
  D) Layer-1 edges: rows gathered directly (no matmul), same window
     aggregation; out = norm_a + norm_b + 2*b1.
"""

import sys

import numpy as np

if "/opt/trn_rl_repo" not in sys.path:
    sys.path.insert(0, "/opt/trn_rl_repo")

import concourse.bacc as bacc
import concourse.bass as bass
import concourse.mybir as mybir
import concourse.tile as tile
from concourse.bass_utils import run_bass_kernel_spmd
from concourse.masks import make_identity

F32 = mybir.dt.float32
F16 = mybir.dt.float16
I16 = mybir.dt.int16
AF = mybir.ActivationFunctionType
ALU = mybir.AluOpType

P = 128
WIN = 32
CPW = 4            # chunks per window (512 edges)
CAP = WIN * 16     # 512 edge capacity per window per type
G = 64             # chunks per gather group
PZ = 3             # chunks per z-matmul psum batch
SB = 8             # windows per stage-flush batch
NEG_SLOPE = 0.2
SEG_EPS = 1e-9
DUMP_ER = -1000.0


def cdiv(a, b):
    return (a + b - 1) // b


# ----------------------------------------------------------------------------
# host-side (integer-only) edge preprocessing
# ----------------------------------------------------------------------------

def pack_windows(dega, degb, nw):
    """Jointly bin-pack dsts into nw windows: <=32 slots, <=CAP edges of
    each type per window. Returns (win_of, slot_of) or None if infeasible."""
    r = len(dega)
    order = np.argsort(-(dega + degb), kind="stable")
    wca = np.zeros(nw, np.int64)
    wcb = np.zeros(nw, np.int64)
    wsl = np.zeros(nw, np.int64)
    win_of = np.full(r, -1, np.int64)
    slot_of = np.full(r, -1, np.int64)
    big = 1 << 40
    for d in order:
        da, db = dega[d], degb[d]
        cost = np.where(
            (wsl < WIN) & (wca + da <= CAP) & (wcb + db <= CAP),
            wca + wcb, big)
        w = int(np.argmin(cost))
        if cost[w] >= big:
            return None
        win_of[d] = w
        slot_of[d] = wsl[w]
        wsl[w] += 1
        wca[w] += da
        wcb[w] += db
    return win_of, slot_of


def edge_slots(src, d_loc, win_of, slot_of, nw):
    """Place edges into (chunk, partition) slots, window-major.

    Returns (srcs, ers_local_pos, cols, valid) each [CH, 128] where CH=nw*CPW.
    ers_local_pos = win*32+slot (row in the per-core er/dst-permuted table),
    -1 on padding. srcs padding = 0."""
    ch = nw * CPW
    w = win_of[d_loc]
    s = slot_of[d_loc]
    order = np.lexsort((s, w))
    src_o, w_o, s_o = src[order], w[order], s[order]
    firsts = np.searchsorted(w_o, np.arange(nw), "left")
    pos = np.arange(len(w_o)) - firsts[w_o]
    assert pos.max(initial=0) < CAP
    chunk = w_o * CPW + pos // P
    part = pos % P
    srcs = np.zeros((ch, P), np.int64)
    ers = np.full((ch, P), -1, np.int64)
    cols = np.zeros((ch, P), np.int64)
    srcs[chunk, part] = src_o
    ers[chunk, part] = w_o * WIN + s_o
    cols[chunk, part] = s_o
    return srcs, ers, cols


def wrap_groups(arr):
    """[CH, 128] int -> [128, CH*8] int16 wrapped per G-chunk group."""
    ch = arr.shape[0]
    out = np.zeros((128, ch * 8), np.int16)
    for g0 in range(0, ch, G):
        gsz = min(G, ch - g0)
        flat = arr[g0:g0 + gsz].reshape(-1)          # i = chunk*128+part
        w16 = flat.reshape(-1, 16).T.astype(np.int16)  # [16, gsz*8]
        out[:, g0 * 8:(g0 + gsz) * 8] = np.tile(w16, (8, 1))
    return out


def block_diag_attn(attn):
    h, d = attn.shape
    out = np.zeros((h * d, h), np.float32)
    for i in range(h):
        out[i * d:(i + 1) * d, i] = attn[i]
    return out


class Cfg:
    def __init__(self, sc_pad, nw0, nw1, n_cores=8, N0=50000, N1=20000,
                 N2=10000, F_in=64, H0=4, D0=32, H1=1, D1=32):
        self.n_cores = n_cores
        self.N0, self.N1, self.N2 = N0, N1, N2
        self.F_in, self.H0, self.D0, self.H1, self.D1 = F_in, H0, D0, H1, D1
        self.R0 = N1 // n_cores
        self.R1 = N2 // n_cores
        self.SC = sc_pad
        self.NW0, self.NW1 = nw0, nw1
        self.R0S = nw0 * WIN
        self.R1S = nw1 * WIN
        self.CH0 = nw0 * CPW
        self.CH1 = nw1 * CPW
        self.DUMP0 = self.R0S
        self.DUMP1 = n_cores * self.R0S
        self.RW0 = H0 * D0 + H0   # 132
        self.RW1 = H1 * D1 + H1   # 33


def prep_all(cfg_dims, feat, src0a, dst0a, src0b, dst0b, src1a, dst1a,
             src1b, dst1b):
    """All integer prep + compacted fp16 feat tables. Returns (cfg, per-core
    host tensors dict list, shared dict)."""
    n_cores, N0, N1, N2, R0, R1 = cfg_dims
    feat = np.asarray(feat, np.float32)

    def split(src, dst, r):
        src, dst = np.asarray(src, np.int64), np.asarray(dst, np.int64)
        out = []
        for c in range(n_cores):
            m = (dst >= c * r) & (dst < (c + 1) * r)
            out.append((src[m], dst[m] - c * r))
        return out

    e0a, e0b = split(src0a, dst0a, R0), split(src0b, dst0b, R0)
    e1a, e1b = split(src1a, dst1a, R1), split(src1b, dst1b, R1)

    def pack_layer(ea, eb, r, nw_start):
        for nw in range(nw_start, nw_start + 6):
            packs = []
            ok = True
            for c in range(n_cores):
                dega = np.bincount(ea[c][1], minlength=r)
                degb = np.bincount(eb[c][1], minlength=r)
                pk = pack_windows(dega, degb, nw)
                if pk is None:
                    ok = False
                    break
                packs.append(pk)
            if ok:
                return nw, packs
        raise RuntimeError("window packing failed")

    nw0, packs0 = pack_layer(e0a, e0b, R0, 82)
    nw1, packs1 = pack_layer(e1a, e1b, R1, 42)

    # per-(core,type) src compaction for layer 0
    srclists = []
    sc_max = 0
    for c in range(n_cores):
        row = []
        for src, _ in (e0a[c], e0b[c]):
            u = np.unique(src)
            sc_max = max(sc_max, len(u))
            row.append(u)
        srclists.append(row)
    sc_pad = cdiv(sc_max, P) * P
    assert sc_pad <= 32767, sc_pad

    cfg = Cfg(sc_pad, nw0, nw1, n_cores, N0, N1, N2)

    # tabrow map: global h id -> z1el table row
    tabrow = np.zeros(N1, np.int64)
    for c in range(n_cores):
        win_of, slot_of = packs0[c]
        pos = win_of * WIN + slot_of                  # [R0]
        tabrow[c * R0 + np.arange(R0)] = c * cfg.R0S + pos

    feat16 = feat.astype(np.float16)

    per_core = []
    perm1_list = []
    for c in range(n_cores):
        m = {}
        win0, slot0 = packs0[c]
        win1, slot1 = packs1[c]
        # layer-0 edge tensors
        for name, (src, d_loc), u in (("e0a", e0a[c], srclists[c][0]),
                                      ("e0b", e0b[c], srclists[c][1])):
            cid = np.searchsorted(u, src)
            srcs, ers, cols = edge_slots(cid, d_loc, win0, slot0, nw0)
            ers[ers < 0] = cfg.DUMP0
            m[f"{name}_g"] = wrap_groups(srcs)
            m[f"{name}_e"] = wrap_groups(ers)
            m[f"{name}_c"] = np.ascontiguousarray(
                cols.T.astype(np.float16))
            ftab = np.zeros((sc_pad, P), np.float16)
            ftab[:len(u), :feat.shape[1]] = feat16[u]
            m[f"{name}_ft"] = ftab
        # layer-1 edge tensors
        for name, (src, d_loc) in (("e1a", e1a[c]), ("e1b", e1b[c])):
            rows = tabrow[src]
            srcs, _, cols = edge_slots(rows, d_loc, win1, slot1, nw1)
            m[f"{name}_g"] = wrap_groups(srcs)
            # er1 row = tabrow of the dst node (dst ids are h ids < N2)
            ed = np.full((cfg.CH1, P), cfg.DUMP1, np.int64)
            w = win1[d_loc]
            s = slot1[d_loc]
            order = np.lexsort((s, w))
            w_o, s_o = w[order], s[order]
            d_o = d_loc[order]
            firsts = np.searchsorted(w_o, np.arange(nw1), "left")
            pos = np.arange(len(w_o)) - firsts[w_o]
            chunk = w_o * CPW + pos // P
            part = pos % P
            ed[chunk, part] = tabrow[c * R1 + d_o]
            m[f"{name}_e"] = wrap_groups(ed)
            m[f"{name}_c"] = np.ascontiguousarray(cols.T.astype(np.float16))
        # dst-permuted featT for er0 build
        ftd = np.zeros((feat.shape[1], cfg.R0S), np.float16)
        pos0 = win0 * WIN + slot0
        ftd[:, pos0] = feat16[c * R0:(c + 1) * R0].T
        m["featTdst"] = ftd
        per_core.append(m)
        # output permutation
        perm1 = np.full(cfg.R1S, -1, np.int64)
        perm1[win1 * WIN + slot1] = np.arange(R1)
        perm1_list.append(perm1)
    return cfg, per_core, perm1_list


# ----------------------------------------------------------------------------
# device program
# ----------------------------------------------------------------------------

def build_program(cfg):
    n_cores = cfg.n_cores
    F_in, H0, D0, H1, D1 = cfg.F_in, cfg.H0, cfg.D0, cfg.H1, cfg.D1
    HD0, HD1 = H0 * D0, H1 * D1
    RW0, RW1 = cfg.RW0, cfg.RW1

    nc = bacc.Bacc("TRN2", target_bir_lowering=False, debug=False,
                   num_devices=n_cores)

    # ---- external inputs --------------------------------------------------
    W0 = nc.dram_tensor("W0", [F_in, HD0], F32, kind="ExternalInput")
    Aler0 = nc.dram_tensor("Aler0", [HD0, 2 * H0], F32, kind="ExternalInput")
    b0r = nc.dram_tensor("b0r", [H0, D0], F32, kind="ExternalInput")
    W1 = nc.dram_tensor("W1", [D0, HD1], F32, kind="ExternalInput")
    Aler1 = nc.dram_tensor("Aler1", [HD1, 2 * H1], F32, kind="ExternalInput")
    b1r = nc.dram_tensor("b1r", [1, D1], F32, kind="ExternalInput")
    iota = nc.dram_tensor("iota", [P, WIN], F16, kind="ExternalInput")
    ones4 = nc.dram_tensor("ones4", [H0, 1], F32, kind="ExternalInput")
    twos = nc.dram_tensor("twos", [1, P], F32, kind="ExternalInput")
    featTdst = nc.dram_tensor("featTdst", [F_in, cfg.R0S], F16,
                              kind="ExternalInput")

    edge_in = {}
    for name, ch in (("e0a", cfg.CH0), ("e0b", cfg.CH0),
                     ("e1a", cfg.CH1), ("e1b", cfg.CH1)):
        d = dict(
            g=nc.dram_tensor(f"{name}_g", [P, ch * 8], I16,
                             kind="ExternalInput"),
            e=nc.dram_tensor(f"{name}_e", [P, ch * 8], I16,
                             kind="ExternalInput"),
            c=nc.dram_tensor(f"{name}_c", [P, ch], F16,
                             kind="ExternalInput"),
            ch=ch,
        )
        if name.startswith("e0"):
            d["ft"] = nc.dram_tensor(f"{name}_ft", [cfg.SC, P], F16,
                                     kind="ExternalInput")
        edge_in[name] = d

    out_t = nc.dram_tensor("out", [cfg.R1S, D1], F32, kind="ExternalOutput")

    with tile.TileContext(nc) as tc:
        from contextlib import ExitStack
        with ExitStack() as ctx:
            const = ctx.enter_context(tc.tile_pool(name="const", bufs=1))
            sbuf = ctx.enter_context(tc.tile_pool(name="sbuf", bufs=3))
            big = ctx.enter_context(tc.tile_pool(name="big", bufs=2))
            stage = ctx.enter_context(tc.tile_pool(name="stage", bufs=2))
            small = ctx.enter_context(tc.tile_pool(name="small", bufs=3))
            psm = ctx.enter_context(tc.tile_pool(name="psm", bufs=1,
                                                 space="PSUM"))
            psz = ctx.enter_context(tc.tile_pool(name="psz", bufs=3,
                                                 space="PSUM"))
            psw = ctx.enter_context(tc.tile_pool(name="psw", bufs=3,
                                                 space="PSUM"))
            dram = ctx.enter_context(tc.tile_pool(name="dram", bufs=1,
                                                  space="DRAM"))

            # ---- internal DRAM ------------------------------------------
            er0_tab = dram.tile([cfg.R0S + P, P], F16)
            h_stage_a = dram.tile([cfg.R0S, HD0], F32)
            h_stage_b = dram.tile([cfg.R0S, HD0], F32)
            z1el_shard = dram.tile([cfg.R0S, RW1 + 1], F16)
            z1el_small = dram.tile([n_cores * cfg.R0S, RW1 + 1], F16,
                                   addr_space="Shared")
            z1el_tab = dram.tile([n_cores * cfg.R0S + P, P], F16)
            o_stage_a = dram.tile([cfg.R1S, D1], F32)
            o_stage_b = dram.tile([cfg.R1S, D1], F32)

            # ---- constants ----------------------------------------------
            ident32 = const.tile([P, P], F32)
            make_identity(nc, ident32[:])
            ident16 = const.tile([P, P], F16)
            nc.vector.tensor_copy(out=ident16[:], in_=ident32[:])
            iota_sb = const.tile([P, WIN], F16)
            nc.sync.dma_start(out=iota_sb[:], in_=iota[:])
            W0_sb = const.tile([F_in, HD0], F32)
            nc.sync.dma_start(out=W0_sb[:], in_=W0[:])
            Aler0_sb = const.tile([HD0, 2 * H0], F32)
            nc.sync.dma_start(out=Aler0_sb[:], in_=Aler0[:])
            b0_sb = const.tile([H0, D0], F32)
            nc.sync.dma_start(out=b0_sb[:], in_=b0r[:])
            W1_sb = const.tile([D0, HD1], F32)
            nc.sync.dma_start(out=W1_sb[:], in_=W1[:])
            Aler1_sb = const.tile([HD1, 2 * H1], F32)
            nc.sync.dma_start(out=Aler1_sb[:], in_=Aler1[:])
            b1_sb = const.tile([1, D1], F32)
            nc.sync.dma_start(out=b1_sb[:], in_=b1r[:])
            ones4_sb = const.tile([H0, 1], F32)
            nc.sync.dma_start(out=ones4_sb[:], in_=ones4[:])
            twos_sb = const.tile([1, P], F32)
            nc.sync.dma_start(out=twos_sb[:], in_=twos[:])

            pt = psm.tile([HD0, F_in], F32, tag="ps_m")
            nc.tensor.transpose(out=pt[:], in_=W0_sb[:],
                                identity=ident32[:F_in, :F_in])
            W0T_sb = const.tile([HD0, F_in], F32)
            nc.scalar.copy(out=W0T_sb[:], in_=pt[:])
            pe = psm.tile([F_in, 2 * H0], F32, tag="ps_m")
            nc.tensor.matmul(out=pe[:], lhsT=W0T_sb[:], rhs=Aler0_sb[:],
                             start=True, stop=True)
            # W0el = [W0 | W0@bd(al0)] f16 ; W0r = W0@bd(ar0) f16
            W0el = const.tile([F_in, RW0], F16)
            nc.vector.tensor_copy(out=W0el[:, :HD0], in_=W0_sb[:])
            nc.vector.tensor_copy(out=W0el[:, HD0:], in_=pe[:, :H0])
            W0r = const.tile([F_in, H0], F16)
            nc.vector.tensor_copy(out=W0r[:], in_=pe[:, H0:])

            pt1 = psm.tile([HD1, D0], F32, tag="ps_m")
            nc.tensor.transpose(out=pt1[:], in_=W1_sb[:],
                                identity=ident32[:D0, :D0])
            W1T_sb = const.tile([HD1, D0], F32)
            nc.scalar.copy(out=W1T_sb[:], in_=pt1[:])
            pe1 = psm.tile([D0, 2 * H1], F32, tag="ps_m")
            nc.tensor.matmul(out=pe1[:], lhsT=W1T_sb[:], rhs=Aler1_sb[:],
                             start=True, stop=True)
            # W1e = [W1 | el col | er col] f16 [D0, 34]
            W1e = const.tile([D0, RW1 + 1], F16)
            nc.vector.tensor_copy(out=W1e[:, :HD1], in_=W1_sb[:])
            nc.vector.tensor_copy(out=W1e[:, HD1:], in_=pe1[:])

            # mb2 = 2*sum_h b0 broadcast [P, D0]; bias1 = 2*b1 [P, D1]
            ps_s = psm.tile([1, D0], F32, tag="ps_m")
            nc.tensor.matmul(out=ps_s[:], lhsT=ones4_sb[:], rhs=b0_sb[:],
                             start=True, stop=True)
            sb0_sb = const.tile([1, D0], F32)
            nc.scalar.copy(out=sb0_sb[:], in_=ps_s[:])
            ps_mb = psm.tile([P, D0], F32, tag="ps_m")
            nc.tensor.matmul(out=ps_mb[:], lhsT=twos_sb[:], rhs=sb0_sb[:],
                             start=True, stop=True)
            mb2_sb = const.tile([P, D0], F32)
            nc.scalar.copy(out=mb2_sb[:], in_=ps_mb[:])
            ps_b1 = psm.tile([P, D1], F32, tag="ps_m")
            nc.tensor.matmul(out=ps_b1[:], lhsT=twos_sb[:], rhs=b1_sb[:],
                             start=True, stop=True)
            bias1_sb = const.tile([P, D1], F32)
            nc.scalar.copy(out=bias1_sb[:], in_=ps_b1[:])

            dump_sb = const.tile([1, H0], F16)
            nc.vector.memset(dump_sb[:], DUMP_ER)
            zdump_sb = const.tile([1, RW1 + 1], F16)
            nc.vector.memset(zdump_sb[:], 0.0)
            nc.vector.memset(zdump_sb[:, RW1:], DUMP_ER)

            # ---- Phase A': er0 table ------------------------------------
            n_dt = cdiv(cfg.R0S, P)
            for t in range(n_dt):
                p = min(P, cfg.R0S - t * P)
                ftd = sbuf.tile([F_in, P], F16, tag="ftd")
                nc.sync.dma_start(out=ftd[:, :p],
                                  in_=featTdst[:, t * P:t * P + p])
                pse = psm.tile([P, H0], F32, tag="ps_m")
                nc.tensor.matmul(out=pse[:p], lhsT=ftd[:, :p], rhs=W0r[:],
                                 start=True, stop=True)
                st = sbuf.tile([P, H0], F16, tag="erst")
                nc.vector.tensor_copy(out=st[:p], in_=pse[:p])
                nc.sync.dma_start(out=er0_tab[t * P:t * P + p, :H0],
                                  in_=st[:p])
            nc.sync.dma_start(out=er0_tab[cfg.DUMP0:cfg.DUMP0 + 1, :H0],
                              in_=dump_sb[:])

            # ---- shared edge-aggregation phase --------------------------
            def edge_phase(name, layer, stage_dram):
                ed = edge_in[name]
                ch = ed["ch"]
                h_, rw = (H0, RW0) if layer == 0 else (H1, RW1)
                hd = h_ * (D0 if layer == 0 else D1)
                nw = cfg.NW0 if layer == 0 else cfg.NW1
                gtab = ed["ft"] if layer == 0 else z1el_tab
                etab = er0_tab if layer == 0 else z1el_tab
                er_col = 0 if layer == 0 else RW1
                stg = {"t": None}

                def flush_stage(w_hi):
                    w_lo = (w_hi // SB) * SB
                    k = w_hi - w_lo + 1
                    nc.sync.dma_start(
                        out=stage_dram[w_lo * WIN:(w_hi + 1) * WIN, :]
                        .rearrange("(j d) f -> d j f", d=WIN),
                        in_=stg["t"][:, :k * hd].rearrange(
                            "d (j f) -> d j f", f=hd))
                    stg["t"] = None

                for g0 in range(0, ch, G):
                    gsz = min(G, ch - g0)
                    ni = gsz * P
                    gidx = sbuf.tile([P, G * 8], I16, tag="gidx")
                    nc.sync.dma_start(out=gidx[:, :gsz * 8],
                                      in_=ed["g"][:, g0 * 8:(g0 + gsz) * 8])
                    eidx = sbuf.tile([P, G * 8], I16, tag="eidx")
                    nc.sync.dma_start(out=eidx[:, :gsz * 8],
                                      in_=ed["e"][:, g0 * 8:(g0 + gsz) * 8])
                    colt = sbuf.tile([P, G], F16, tag="colt")
                    nc.sync.dma_start(out=colt[:, :gsz],
                                      in_=ed["c"][:, g0:g0 + gsz])
                    erg = big.tile([P, G, P], F16, tag="erg")
                    nc.gpsimd.dma_gather(
                        erg[:, :gsz, :], etab[:], eidx[:, :gsz * 8],
                        ni, ni, P, single_packet=False)
                    cmp = big.tile([P, G, WIN], F16, tag="cmp")
                    nc.vector.tensor_tensor(
                        out=cmp[:, :gsz],
                        in0=colt[:, :gsz].unsqueeze(2).to_broadcast(
                            [P, gsz, WIN]),
                        in1=iota_sb[:].unsqueeze(1).to_broadcast(
                            [P, gsz, WIN]),
                        op=ALU.is_equal)
                    zx = big.tile([P, G, rw], F16, tag=f"zx{layer}")
                    if layer == 0:
                        featg = big.tile([P, 1, G * P], F16, tag="featg")
                        nc.gpsimd.dma_gather(
                            featg[:, :, :ni], gtab[:], gidx[:, :gsz * 8],
                            ni, ni, P, transpose=True, single_packet=False)
                        for j0 in range(0, gsz, PZ):
                            bs = min(PZ, gsz - j0)
                            zps = psz.tile([P, PZ, RW0], F32, tag="ps_z")
                            for dj in range(bs):
                                nc.tensor.matmul(
                                    out=zps[:, dj, :],
                                    lhsT=featg[:F_in, 0,
                                               (j0 + dj) * P:(j0 + dj + 1) * P],
                                    rhs=W0el[:], start=True, stop=True)
                            lg = small.tile([P, PZ, h_], F32, tag="lg")
                            nc.vector.tensor_tensor(
                                out=lg[:, :bs], in0=zps[:, :bs, hd:hd + h_],
                                in1=erg[:, j0:j0 + bs, :h_], op=ALU.add)
                            lk = small.tile([P, PZ, h_], F32, tag="lk")
                            nc.vector.tensor_scalar(
                                out=lk[:, :bs], in0=lg[:, :bs],
                                scalar1=NEG_SLOPE, scalar2=None, op0=ALU.mult)
                            nc.vector.tensor_tensor(
                                out=lg[:, :bs], in0=lg[:, :bs],
                                in1=lk[:, :bs], op=ALU.max)
                            ex = small.tile([P, PZ, h_], F16, tag="ex")
                            nc.scalar.activation(
                                out=ex[:, :bs], in_=lg[:, :bs], func=AF.Exp)
                            nc.vector.tensor_tensor(
                                out=zx[:, j0:j0 + bs, :hd].rearrange(
                                    "p j (h d) -> p j h d", h=h_),
                                in0=zps[:, :bs, :hd].rearrange(
                                    "p j (h d) -> p j h d", h=h_),
                                in1=ex[:, :bs].unsqueeze(3).to_broadcast(
                                    [P, bs, h_, D0]),
                                op=ALU.mult)
                            nc.vector.tensor_copy(
                                out=zx[:, j0:j0 + bs, hd:hd + h_],
                                in_=ex[:, :bs])
                    else:
                        rowg = big.tile([P, G, P], F16, tag="rowg")
                        nc.gpsimd.dma_gather(
                            rowg[:, :gsz, :], gtab[:], gidx[:, :gsz * 8],
                            ni, ni, P, single_packet=False)
                        lg = sbuf.tile([P, G, 1], F32, tag="lg1")
                        nc.vector.tensor_tensor(
                            out=lg[:, :gsz], in0=rowg[:, :gsz, hd:hd + 1],
                            in1=erg[:, :gsz, er_col:er_col + 1], op=ALU.add)
                        lk = sbuf.tile([P, G, 1], F32, tag="lk1")
                        nc.vector.tensor_scalar(
                            out=lk[:, :gsz], in0=lg[:, :gsz],
                            scalar1=NEG_SLOPE, scalar2=None, op0=ALU.mult)
                        nc.vector.tensor_tensor(
                            out=lg[:, :gsz], in0=lg[:, :gsz], in1=lk[:, :gsz],
                            op=ALU.max)
                        ex = sbuf.tile([P, G, 1], F16, tag="ex1")
                        nc.scalar.activation(
                            out=ex[:, :gsz], in_=lg[:, :gsz], func=AF.Exp)
                        nc.vector.tensor_tensor(
                            out=zx[:, :gsz, :hd], in0=rowg[:, :gsz, :hd],
                            in1=ex[:, :gsz].to_broadcast([P, gsz, hd]),
                            op=ALU.mult)
                        nc.vector.tensor_copy(
                            out=zx[:, :gsz, hd:hd + 1], in_=ex[:, :gsz])

                    for j in range(gsz):
                        chn = g0 + j
                        w = chn // CPW
                        first = chn % CPW == 0
                        last = chn % CPW == CPW - 1
                        if first:
                            pw = psw.tile([WIN, rw], F32, tag="ps_w",
                                          name=f"pw_{name}_{w}")
                            psw_cur[0] = pw
                        nc.tensor.matmul(
                            out=psw_cur[0][:], lhsT=cmp[:, j, :],
                            rhs=zx[:, j, :], start=first, stop=last)
                        if last:
                            pw = psw_cur[0]
                            sm = small.tile([WIN, h_], F32, tag="sm")
                            nc.vector.tensor_scalar(
                                out=sm[:], in0=pw[:, hd:hd + h_],
                                scalar1=SEG_EPS, scalar2=None, op0=ALU.max)
                            rs = small.tile([WIN, h_], F32, tag="rs")
                            nc.vector.reciprocal(out=rs[:], in_=sm[:])
                            if stg["t"] is None:
                                stg["t"] = stage.tile(
                                    [WIN, SB * hd], F32, tag="hstg",
                                    name=f"stg_{name}_{w}")
                            slot = w % SB
                            dstv = stg["t"][:, slot * hd:(slot + 1) * hd] \
                                .rearrange("d (h f) -> d h f", h=h_)
                            nc.vector.tensor_tensor(
                                out=dstv,
                                in0=pw[:, :hd].rearrange(
                                    "d (h f) -> d h f", h=h_),
                                in1=rs[:].unsqueeze(2).to_broadcast(
                                    [WIN, h_, hd // h_]),
                                op=ALU.mult)
                            if slot == SB - 1 or w == nw - 1:
                                flush_stage(w)

            psw_cur = [None]

            # ---- Phase B: layer-0 edges ---------------------------------
            edge_phase("e0a", 0, h_stage_a)
            edge_phase("e0b", 0, h_stage_b)

            # ---- Phase C: h build + z1el shard + AllGather --------------
            n_ht = cdiv(cfg.R0S, P)
            for i in range(n_ht):
                p = min(P, cfg.R0S - i * P)
                at = sbuf.tile([P, HD0], F32, tag="ha")
                bt = sbuf.tile([P, HD0], F32, tag="hb")
                nc.sync.dma_start(out=at[:p], in_=h_stage_a[i * P:i * P + p, :])
                nc.sync.dma_start(out=bt[:p], in_=h_stage_b[i * P:i * P + p, :])
                nc.vector.tensor_add(out=at[:p], in0=at[:p], in1=bt[:p])
                hs = sbuf.tile([P, D0], F32, tag="hs")
                nc.vector.tensor_add(out=hs[:p], in0=at[:p, 0:D0],
                                     in1=at[:p, D0:2 * D0])
                for h in range(2, H0):
                    nc.vector.tensor_add(
                        out=hs[:p], in0=hs[:p],
                        in1=at[:p, h * D0:(h + 1) * D0])
                nc.vector.tensor_add(out=hs[:p], in0=hs[:p], in1=mb2_sb[:p])
                hr = sbuf.tile([P, D0], F16, tag="hr")
                nc.scalar.activation(out=hr[:p], in_=hs[:p], func=AF.Relu,
                                     scale=1.0 / H0)
                htp = psm.tile([D0, P], F16, tag="ps_m16")
                nc.tensor.transpose(out=htp[:, :p], in_=hr[:p],
                                    identity=ident16[:p, :p])
                hts = sbuf.tile([D0, P], F16, tag="hts")
                nc.vector.tensor_copy(out=hts[:, :p], in_=htp[:, :p])
                zp1 = psm.tile([P, RW1 + 1], F32, tag="ps_m")
                nc.tensor.matmul(out=zp1[:p], lhsT=hts[:, :p], rhs=W1e[:],
                                 start=True, stop=True)
                z1s = sbuf.tile([P, RW1 + 1], F16, tag="z1s")
                nc.vector.tensor_copy(out=z1s[:p], in_=zp1[:p])
                nc.sync.dma_start(out=z1el_shard[i * P:i * P + p, :],
                                  in_=z1s[:p])

            nc.gpsimd.collective_compute(
                "AllGather", ALU.bypass,
                replica_groups=[list(range(n_cores))],
                ins=[z1el_shard.opt()], outs=[z1el_small.opt()])

            # restride compact rows into 256B gather rows + dump row
            nc.sync.dma_start(out=z1el_tab[:n_cores * cfg.R0S, :RW1 + 1],
                              in_=z1el_small[:])
            nc.sync.dma_start(out=z1el_tab[cfg.DUMP1:cfg.DUMP1 + 1, :RW1 + 1],
                              in_=zdump_sb[:])

            # ---- Phase D: layer-1 edges ---------------------------------
            edge_phase("e1a", 1, o_stage_a)
            edge_phase("e1b", 1, o_stage_b)

            # ---- final combine ------------------------------------------
            n_ot = cdiv(cfg.R1S, P)
            for i in range(n_ot):
                p = min(P, cfg.R1S - i * P)
                oa = sbuf.tile([P, D1], F32, tag="oa")
                ob = sbuf.tile([P, D1], F32, tag="ob")
                nc.sync.dma_start(out=oa[:p], in_=o_stage_a[i * P:i * P + p, :])
                nc.sync.dma_start(out=ob[:p], in_=o_stage_b[i * P:i * P + p, :])
                nc.vector.tensor_add(out=oa[:p], in0=oa[:p], in1=ob[:p])
                nc.vector.tensor_add(out=oa[:p], in0=oa[:p], in1=bias1_sb[:p])
                nc.sync.dma_start(out=out_t[i * P:i * P + p, :], in_=oa[:p])

    nc.compile()
    return nc


# ----------------------------------------------------------------------------
# host driver
# ----------------------------------------------------------------------------

_CACHED = {}


def kernel(**inputs):
    dims = (8, 50000, 20000, 10000, 2500, 1250)
    cfg, per_core, perm1_list = prep_all(
        dims, inputs["feat"], inputs["src0a"], inputs["dst0a"],
        inputs["src0b"], inputs["dst0b"], inputs["src1a"], inputs["dst1a"],
        inputs["src1b"], inputs["dst1b"])

    key = (cfg.SC, cfg.NW0, cfg.NW1)
    if key not in _CACHED:
        _CACHED[key] = build_program(cfg)
    nc = _CACHED[key]

    shared = dict(
        W0=np.asarray(inputs["W0"], np.float32),
        Aler0=np.concatenate(
            [block_diag_attn(np.asarray(inputs["attn_l0"], np.float32)),
             block_diag_attn(np.asarray(inputs["attn_r0"], np.float32))],
            axis=1),
        b0r=np.asarray(inputs["b0"], np.float32).reshape(cfg.H0, cfg.D0),
        W1=np.asarray(inputs["W1"], np.float32),
        Aler1=np.concatenate(
            [block_diag_attn(np.asarray(inputs["attn_l1"], np.float32)),
             block_diag_attn(np.asarray(inputs["attn_r1"], np.float32))],
            axis=1),
        b1r=np.asarray(inputs["b1"], np.float32).reshape(1, cfg.D1),
        iota=np.tile(np.arange(WIN, dtype=np.float16), (P, 1)),
        ones4=np.ones((cfg.H0, 1), np.float32),
        twos=np.full((1, P), 2.0, np.float32),
    )
    in_maps = []
    for c in range(cfg.n_cores):
        m = dict(shared)
        m.update(per_core[c])
        in_maps.append(m)

    res = run_bass_kernel_spmd(nc, in_maps, list(range(cfg.n_cores)))

    full = np.zeros((cfg.N2, cfg.D1), np.float32)
    for c in range(cfg.n_cores):
        o = res.results[c]["out"]
        perm1 = perm1_list[c]
        v = perm1 >= 0
        full[c * cfg.R1 + perm1[v]] = o[v]
    return full.reshape(cfg.N2, cfg.H1, cfg.D1).astype(np.float32)


# revision 21
# speedup vs baseline: 2.4715x; 2.4715x over previous
"""2-layer multi-edge-type GAT on Trainium2, 8-core SPMD (v3).

Key bottleneck on TRN2: gpsimd SWDGE descriptor generation runs at ~8-10 ns
per gathered row, so per-edge gathers cost ~1 ms/layer. This version
eliminates ALL layer-0 gathers: the edge list is known host-side, so the
host stages edge-ordered transposed feat blocks (pure data movement, no
flops) that the device reads with dense DMAs. Layer-1 sources reference the
device-computed h, so one batched int16 dma_gather per 64-chunk group
remains (the only per-edge descriptors in the kernel).

Structure per core (edges sharded by dst owner; dst nodes bin-packed into
32-slot windows jointly balanced over both edge types, <=512 edges/type per
window = exactly 4 chunks of 128):
  A) er0 window table: er = (feat @ W0)*attn_r for the permuted dst list,
     laid out [32 slots, NW0*H] for per-window matmul access.
  B) Layer-0 edges: per chunk, fp16 matmul feat_edges.T @ [W0|W0*attn_l]
     gives [z|el] in PSUM; a second small matmul cmpT.T @ er_win adds er;
     DVE: ex = exp(leakyrelu(el+er)) and rows = [ex*z | ex] fp16; one fp16
     one-hot window matmul accumulates [num|den] per window in PSUM;
     window flush normalizes (softmax shift dropped: logits O(1)).
  C) h = relu(mean_h(gat_a+gat_b)); z1el rows [z1|el1|er1] AllGathered
     compact, restrided to 256B rows for gathers.
  D) Layer-1 edges: batched dma_gather of [z1|el1] rows by src; er1 per
     edge via DVE multiply-reduce of the one-hot against window er values;
     same window aggregation; out = norm_a + norm_b + 2*b1.

Padding edges use zero feat blocks / a zero table row and all-zero one-hot
columns, so they contribute exactly nothing (ex stays finite, fp16-safe).
"""

import sys

import numpy as np

if "/opt/trn_rl_repo" not in sys.path:
    sys.path.insert(0, "/opt/trn_rl_repo")

import concourse.bacc as bacc
import concourse.bass as bass
import concourse.mybir as mybir
import concourse.tile as tile
from concourse.bass_utils import run_bass_kernel_spmd
from concourse.masks import make_identity

F32 = mybir.dt.float32
F16 = mybir.dt.float16
I16 = mybir.dt.int16
AF = mybir.ActivationFunctionType
ALU = mybir.AluOpType

P = 128
WIN = 32
CPW = 4            # chunks per window (512 edges)
CAP = WIN * 16     # 512-edge capacity per window per type
G = 64             # chunks per group
PZ = 3             # chunks per z-matmul psum batch
SB = 8             # windows per stage-flush batch
NEG_SLOPE = 0.2
SEG_EPS = 1e-9


def cdiv(a, b):
    return (a + b - 1) // b


# ----------------------------------------------------------------------------
# host-side (integer-only / data-movement-only) preprocessing
# ----------------------------------------------------------------------------

def pack_windows(dega, degb, nw):
    """Jointly bin-pack dsts into nw windows: <=32 slots, <=CAP edges of
    each type. Returns (win_of, slot_of) or None if infeasible."""
    r = len(dega)
    order = np.argsort(-(dega + degb), kind="stable")
    wca = np.zeros(nw, np.int64)
    wcb = np.zeros(nw, np.int64)
    wsl = np.zeros(nw, np.int64)
    win_of = np.full(r, -1, np.int64)
    slot_of = np.full(r, -1, np.int64)
    big = 1 << 40
    for d in order:
        da, db = dega[d], degb[d]
        cost = np.where(
            (wsl < WIN) & (wca + da <= CAP) & (wcb + db <= CAP),
            wca + wcb, big)
        w = int(np.argmin(cost))
        if cost[w] >= big:
            return None
        win_of[d] = w
        slot_of[d] = wsl[w]
        wsl[w] += 1
        wca[w] += da
        wcb[w] += db
    return win_of, slot_of


def edge_slots(src, d_loc, win_of, slot_of, nw):
    """Place edges into (chunk, partition) slots, window-major.
    Returns (srcs, cols): [CH, 128]; padding src=-1, col=-1."""
    ch = nw * CPW
    w = win_of[d_loc]
    s = slot_of[d_loc]
    order = np.lexsort((s, w))
    src_o, w_o, s_o = src[order], w[order], s[order]
    firsts = np.searchsorted(w_o, np.arange(nw), "left")
    pos = np.arange(len(w_o)) - firsts[w_o]
    assert pos.max(initial=0) < CAP
    chunk = w_o * CPW + pos // P
    part = pos % P
    srcs = np.full((ch, P), -1, np.int64)
    cols = np.full((ch, P), -1, np.int64)
    srcs[chunk, part] = src_o
    cols[chunk, part] = s_o
    return srcs, cols


def one_hots(cols):
    """cols [CH, 128] -> cm [128, CH*32] f16 and ct [32, CH*128] f16."""
    ch = cols.shape[0]
    rng32 = np.arange(WIN)
    oh = (cols[:, :, None] == rng32[None, None, :])        # [CH, 128, 32]
    cm = np.ascontiguousarray(
        oh.transpose(1, 0, 2).reshape(P, ch * WIN)).astype(np.float16)
    ct = np.ascontiguousarray(
        oh.transpose(2, 0, 1).reshape(WIN, ch * P)).astype(np.float16)
    return cm, ct


def wrap_groups(arr, group):
    """[CH, 128] int -> [128, CH*8] int16 wrapped per `group`-chunk group."""
    ch = arr.shape[0]
    out = np.zeros((128, ch * 8), np.int16)
    for g0 in range(0, ch, group):
        gsz = min(group, ch - g0)
        flat = arr[g0:g0 + gsz].reshape(-1)
        w16 = flat.reshape(-1, 16).T.astype(np.int16)
        out[:, g0 * 8:(g0 + gsz) * 8] = np.tile(w16, (8, 1))
    return out


def block_diag_attn(attn):
    h, d = attn.shape
    out = np.zeros((h * d, h), np.float32)
    for i in range(h):
        out[i * d:(i + 1) * d, i] = attn[i]
    return out


class Cfg:
    def __init__(self, nw0, nw1, n_cores=8, N0=50000, N1=20000,
                 N2=10000, F_in=64, H0=4, D0=32, H1=1, D1=32):
        self.n_cores = n_cores
        self.N0, self.N1, self.N2 = N0, N1, N2
        self.F_in, self.H0, self.D0, self.H1, self.D1 = F_in, H0, D0, H1, D1
        self.R0 = N1 // n_cores
        self.R1 = N2 // n_cores
        self.NW0, self.NW1 = nw0, nw1
        self.R0S = nw0 * WIN
        self.R1S = nw1 * WIN
        self.CH0 = nw0 * CPW
        self.CH1 = nw1 * CPW
        self.RW0 = H0 * D0 + H0   # 132
        self.RW1 = H1 * D1 + H1   # 33
        self.NTAB = n_cores * self.R0S   # z1el table rows; +1 zero row
        self.ZROW = self.NTAB
        self.NER1 = cdiv(self.R1S, P) * P


def prep_all(dims, feat, src0a, dst0a, src0b, dst0b, src1a, dst1a,
             src1b, dst1b):
    n_cores, N0, N1, N2, R0, R1 = dims
    feat = np.asarray(feat, np.float32)
    feat16 = feat.astype(np.float16)
    feat16z = np.vstack([feat16, np.zeros((1, feat.shape[1]), np.float16)])

    def split(src, dst, r):
        src, dst = np.asarray(src, np.int64), np.asarray(dst, np.int64)
        out = []
        for c in range(n_cores):
            m = (dst >= c * r) & (dst < (c + 1) * r)
            out.append((src[m], dst[m] - c * r))
        return out

    e0a, e0b = split(src0a, dst0a, R0), split(src0b, dst0b, R0)
    e1a, e1b = split(src1a, dst1a, R1), split(src1b, dst1b, R1)

    def pack_layer(ea, eb, r, nw_start):
        for nw in range(nw_start, nw_start + 6):
            packs = []
            ok = True
            for c in range(n_cores):
                dega = np.bincount(ea[c][1], minlength=r)
                degb = np.bincount(eb[c][1], minlength=r)
                pk = pack_windows(dega, degb, nw)
                if pk is None:
                    ok = False
                    break
                packs.append(pk)
            if ok:
                return nw, packs
        raise RuntimeError("window packing failed")

    nw0, packs0 = pack_layer(e0a, e0b, R0, 82)
    nw1, packs1 = pack_layer(e1a, e1b, R1, 42)
    cfg = Cfg(nw0, nw1, n_cores, N0, N1, N2)

    tabrow = np.zeros(N1, np.int64)
    for c in range(n_cores):
        win_of, slot_of = packs0[c]
        tabrow[c * R0 + np.arange(R0)] = c * cfg.R0S + win_of * WIN + slot_of

    per_core = []
    perm1_list = []
    for c in range(n_cores):
        m = {}
        win0, slot0 = packs0[c]
        win1, slot1 = packs1[c]
        for name, (src, d_loc) in (("e0a", e0a[c]), ("e0b", e0b[c])):
            srcs, cols = edge_slots(src, d_loc, win0, slot0, nw0)
            srcs[srcs < 0] = N0                    # zero feat row
            fe = feat16z[srcs.reshape(-1)].reshape(cfg.CH0, P, cfg.F_in)
            m[f"{name}_fe"] = np.ascontiguousarray(
                fe.transpose(2, 0, 1).reshape(cfg.F_in, cfg.CH0 * P))
            cm, ct = one_hots(cols)
            m[f"{name}_cm"] = cm
            m[f"{name}_ct"] = ct
        for name, (src, d_loc) in (("e1a", e1a[c]), ("e1b", e1b[c])):
            rows = tabrow[src]
            srcs, cols = edge_slots(rows, d_loc, win1, slot1, nw1)
            srcs[srcs < 0] = cfg.ZROW
            m[f"{name}_g"] = wrap_groups(srcs, G)
            cm, ct = one_hots(cols)
            m[f"{name}_cm"] = cm
            m[f"{name}_ct"] = ct
        # dst-permuted featT for er0 build (empty slots zero)
        ftd = np.zeros((cfg.F_in, cfg.R0S), np.float16)
        ftd[:, win0 * WIN + slot0] = feat16[c * R0:(c + 1) * R0].T
        m["featTdst"] = ftd
        # er1 window-value gather rows: slot i=(w*32+s) -> tabrow(dst)
        er1r = np.full(cfg.NER1, cfg.ZROW, np.int64)
        er1r[win1 * WIN + slot1] = tabrow[c * R1 + np.arange(R1)]
        m["er1rows"] = wrap_groups(er1r.reshape(-1, P), cdiv(cfg.NER1, P))
        per_core.append(m)
        perm1 = np.full(cfg.R1S, -1, np.int64)
        perm1[win1 * WIN + slot1] = np.arange(R1)
        perm1_list.append(perm1)
    return cfg, per_core, perm1_list


# ----------------------------------------------------------------------------
# device program
# ----------------------------------------------------------------------------

def build_program(cfg):
    n_cores = cfg.n_cores
    F_in, H0, D0, H1, D1 = cfg.F_in, cfg.H0, cfg.D0, cfg.H1, cfg.D1
    HD0, HD1 = H0 * D0, H1 * D1
    RW0, RW1 = cfg.RW0, cfg.RW1
    NQ = cfg.NER1 // P

    nc = bacc.Bacc("TRN2", target_bir_lowering=False, debug=False,
                   num_devices=n_cores)

    W0 = nc.dram_tensor("W0", [F_in, HD0], F32, kind="ExternalInput")
    Aler0 = nc.dram_tensor("Aler0", [HD0, 2 * H0], F32, kind="ExternalInput")
    b0r = nc.dram_tensor("b0r", [H0, D0], F32, kind="ExternalInput")
    W1 = nc.dram_tensor("W1", [D0, HD1], F32, kind="ExternalInput")
    Aler1 = nc.dram_tensor("Aler1", [HD1, 2 * H1], F32, kind="ExternalInput")
    b1r = nc.dram_tensor("b1r", [1, D1], F32, kind="ExternalInput")
    ones4 = nc.dram_tensor("ones4", [H0, 1], F32, kind="ExternalInput")
    twos = nc.dram_tensor("twos", [1, P], F32, kind="ExternalInput")
    featTdst = nc.dram_tensor("featTdst", [F_in, cfg.R0S], F16,
                              kind="ExternalInput")
    er1rows = nc.dram_tensor("er1rows", [P, NQ * 8], I16,
                             kind="ExternalInput")

    edge_in = {}
    for name in ("e0a", "e0b"):
        edge_in[name] = dict(
            fe=nc.dram_tensor(f"{name}_fe", [F_in, cfg.CH0 * P], F16,
                              kind="ExternalInput"),
            cm=nc.dram_tensor(f"{name}_cm", [P, cfg.CH0 * WIN], F16,
                              kind="ExternalInput"),
            ct=nc.dram_tensor(f"{name}_ct", [WIN, cfg.CH0 * P], F16,
                              kind="ExternalInput"),
            ch=cfg.CH0,
        )
    for name in ("e1a", "e1b"):
        edge_in[name] = dict(
            g=nc.dram_tensor(f"{name}_g", [P, cfg.CH1 * 8], I16,
                             kind="ExternalInput"),
            cm=nc.dram_tensor(f"{name}_cm", [P, cfg.CH1 * WIN], F16,
                              kind="ExternalInput"),
            ct=nc.dram_tensor(f"{name}_ct", [WIN, cfg.CH1 * P], F16,
                              kind="ExternalInput"),
            ch=cfg.CH1,
        )

    out_t = nc.dram_tensor("out", [cfg.R1S, D1], F32, kind="ExternalOutput")

    with tile.TileContext(nc) as tc:
        from contextlib import ExitStack
        with ExitStack() as ctx:
            const = ctx.enter_context(tc.tile_pool(name="const", bufs=1))
            sbuf = ctx.enter_context(tc.tile_pool(name="sbuf", bufs=3))
            big = ctx.enter_context(tc.tile_pool(name="big", bufs=2))
            stage = ctx.enter_context(tc.tile_pool(name="stage", bufs=2))
            small = ctx.enter_context(tc.tile_pool(name="small", bufs=3))
            psm = ctx.enter_context(tc.tile_pool(name="psm", bufs=1,
                                                 space="PSUM"))
            psz = ctx.enter_context(tc.tile_pool(name="psz", bufs=3,
                                                 space="PSUM"))
            psw = ctx.enter_context(tc.tile_pool(name="psw", bufs=3,
                                                 space="PSUM"))
            dram = ctx.enter_context(tc.tile_pool(name="dram", bufs=1,
                                                  space="DRAM"))

            # ---- internal DRAM ------------------------------------------
            erl0_dram = dram.tile([WIN, cfg.NW0 * H0], F16)
            h_stage_a = dram.tile([cfg.R0S, HD0], F32)
            h_stage_b = dram.tile([cfg.R0S, HD0], F32)
            z1el_shard = dram.tile([cfg.R0S, RW1 + 1], F16)
            z1el_small = dram.tile([cfg.NTAB, RW1 + 1], F16,
                                   addr_space="Shared")
            z1el_tab = dram.tile([cfg.NTAB + P, P], F16)
            o_stage_a = dram.tile([cfg.R1S, D1], F32)
            o_stage_b = dram.tile([cfg.R1S, D1], F32)

            # ---- constants ----------------------------------------------
            ident32 = const.tile([P, P], F32)
            make_identity(nc, ident32[:])
            ident16 = const.tile([P, P], F16)
            nc.vector.tensor_copy(out=ident16[:], in_=ident32[:])
            W0_sb = const.tile([F_in, HD0], F32)
            nc.sync.dma_start(out=W0_sb[:], in_=W0[:])
            Aler0_sb = const.tile([HD0, 2 * H0], F32)
            nc.sync.dma_start(out=Aler0_sb[:], in_=Aler0[:])
            b0_sb = const.tile([H0, D0], F32)
            nc.sync.dma_start(out=b0_sb[:], in_=b0r[:])
            W1_sb = const.tile([D0, HD1], F32)
            nc.sync.dma_start(out=W1_sb[:], in_=W1[:])
            Aler1_sb = const.tile([HD1, 2 * H1], F32)
            nc.sync.dma_start(out=Aler1_sb[:], in_=Aler1[:])
            b1_sb = const.tile([1, D1], F32)
            nc.sync.dma_start(out=b1_sb[:], in_=b1r[:])
            ones4_sb = const.tile([H0, 1], F32)
            nc.sync.dma_start(out=ones4_sb[:], in_=ones4[:])
            twos_sb = const.tile([1, P], F32)
            nc.sync.dma_start(out=twos_sb[:], in_=twos[:])

            pt = psm.tile([HD0, F_in], F32, tag="ps_m")
            nc.tensor.transpose(out=pt[:], in_=W0_sb[:],
                                identity=ident32[:F_in, :F_in])
            W0T_sb = const.tile([HD0, F_in], F32)
            nc.scalar.copy(out=W0T_sb[:], in_=pt[:])
            pe = psm.tile([F_in, 2 * H0], F32, tag="ps_m")
            nc.tensor.matmul(out=pe[:], lhsT=W0T_sb[:], rhs=Aler0_sb[:],
                             start=True, stop=True)
            W0el = const.tile([F_in, RW0], F16)
            nc.vector.tensor_copy(out=W0el[:, :HD0], in_=W0_sb[:])
            nc.vector.tensor_copy(out=W0el[:, HD0:], in_=pe[:, :H0])
            W0r = const.tile([F_in, H0], F16)
            nc.vector.tensor_copy(out=W0r[:], in_=pe[:, H0:])

            pt1 = psm.tile([HD1, D0], F32, tag="ps_m")
            nc.tensor.transpose(out=pt1[:], in_=W1_sb[:],
                                identity=ident32[:D0, :D0])
            W1T_sb = const.tile([HD1, D0], F32)
            nc.scalar.copy(out=W1T_sb[:], in_=pt1[:])
            pe1 = psm.tile([D0, 2 * H1], F32, tag="ps_m")
            nc.tensor.matmul(out=pe1[:], lhsT=W1T_sb[:], rhs=Aler1_sb[:],
                             start=True, stop=True)
            W1e = const.tile([D0, RW1 + 1], F16)
            nc.vector.tensor_copy(out=W1e[:, :HD1], in_=W1_sb[:])
            nc.vector.tensor_copy(out=W1e[:, HD1:], in_=pe1[:])

            ps_s = psm.tile([1, D0], F32, tag="ps_m")
            nc.tensor.matmul(out=ps_s[:], lhsT=ones4_sb[:], rhs=b0_sb[:],
                             start=True, stop=True)
            sb0_sb = const.tile([1, D0], F32)
            nc.scalar.copy(out=sb0_sb[:], in_=ps_s[:])
            ps_mb = psm.tile([P, D0], F32, tag="ps_m")
            nc.tensor.matmul(out=ps_mb[:], lhsT=twos_sb[:], rhs=sb0_sb[:],
                             start=True, stop=True)
            mb2_sb = const.tile([P, D0], F32)
            nc.scalar.copy(out=mb2_sb[:], in_=ps_mb[:])
            ps_b1 = psm.tile([P, D1], F32, tag="ps_m")
            nc.tensor.matmul(out=ps_b1[:], lhsT=twos_sb[:], rhs=b1_sb[:],
                             start=True, stop=True)
            bias1_sb = const.tile([P, D1], F32)
            nc.scalar.copy(out=bias1_sb[:], in_=ps_b1[:])

            zrow_sb = const.tile([1, RW1 + 1], F16)
            nc.vector.memset(zrow_sb[:], 0.0)

            # ---- Phase A': er0 window table -----------------------------
            n_dt = cdiv(cfg.R0S, P)
            for t in range(n_dt):
                p = min(P, cfg.R0S - t * P)
                q = p // WIN
                ftd = sbuf.tile([F_in, P], F16, tag="ftd")
                nc.sync.dma_start(out=ftd[:, :p],
                                  in_=featTdst[:, t * P:t * P + p])
                pse = psm.tile([P, H0], F32, tag="ps_m")
                nc.tensor.matmul(out=pse[:p], lhsT=ftd[:, :p], rhs=W0r[:],
                                 start=True, stop=True)
                st = sbuf.tile([P, H0], F16, tag="erst")
                nc.vector.tensor_copy(out=st[:p], in_=pse[:p])
                for qq in range(q):
                    nc.sync.dma_start(
                        out=erl0_dram[:, (4 * t + qq) * H0:
                                      (4 * t + qq + 1) * H0],
                        in_=st[qq * WIN:(qq + 1) * WIN, :])
            erl0 = const.tile([WIN, cfg.NW0 * H0], F16)
            nc.sync.dma_start(out=erl0[:], in_=erl0_dram[:])

            # ---- edge aggregation ---------------------------------------
            psw_cur = [None]

            def edge_phase(name, layer, stage_dram, erl1=None):
                ed = edge_in[name]
                ch = ed["ch"]
                h_, rw = (H0, RW0) if layer == 0 else (H1, RW1)
                hd = h_ * (D0 if layer == 0 else D1)
                nw = cfg.NW0 if layer == 0 else cfg.NW1
                stg = {"t": None}

                def flush_stage(w_hi):
                    w_lo = (w_hi // SB) * SB
                    k = w_hi - w_lo + 1
                    nc.sync.dma_start(
                        out=stage_dram[w_lo * WIN:(w_hi + 1) * WIN, :]
                        .rearrange("(j d) f -> d j f", d=WIN),
                        in_=stg["t"][:, :k * hd].rearrange(
                            "d (j f) -> d j f", f=hd))
                    stg["t"] = None

                for g0 in range(0, ch, G):
                    gsz = min(G, ch - g0)
                    ni = gsz * P
                    cm = big.tile([P, G * WIN], F16, tag="cm")
                    nc.sync.dma_start(
                        out=cm[:, :gsz * WIN],
                        in_=ed["cm"][:, g0 * WIN:(g0 + gsz) * WIN])
                    zx = big.tile([P, G, RW0], F16, tag="zx")
                    gb = big.tile([P, G * P], F16, tag="gbig")

                    if layer == 0:
                        fe = gb
                        nc.sync.dma_start(
                            out=fe[:F_in, :ni],
                            in_=ed["fe"][:, g0 * P:(g0 + gsz) * P])
                        ct = big.tile([WIN, G * P], F16, tag="ct")
                        nc.sync.dma_start(
                            out=ct[:, :ni],
                            in_=ed["ct"][:, g0 * P:(g0 + gsz) * P])
                        for j0 in range(0, gsz, PZ):
                            bs = min(PZ, gsz - j0)
                            zps = psz.tile([P, PZ, RW0], F32, tag="ps_z")
                            for dj in range(bs):
                                j = j0 + dj
                                nc.tensor.matmul(
                                    out=zps[:, dj, :],
                                    lhsT=fe[:F_in, j * P:(j + 1) * P],
                                    rhs=W0el[:], start=True, stop=True)
                                w = (g0 + j) // CPW
                                # accumulate er onto the el columns
                                nc.tensor.matmul(
                                    out=zps[:, dj, HD0:RW0],
                                    lhsT=ct[:, j * P:(j + 1) * P],
                                    rhs=erl0[:, w * H0:(w + 1) * H0],
                                    start=False, stop=True)
                            lg = small.tile([P, PZ, H0], F32, tag="lg")
                            nc.vector.tensor_copy(
                                out=lg[:, :bs], in_=zps[:, :bs, HD0:RW0])
                            nc.vector.scalar_tensor_tensor(
                                out=lg[:, :bs], in0=lg[:, :bs],
                                scalar=NEG_SLOPE, in1=lg[:, :bs],
                                op0=ALU.mult, op1=ALU.max)
                            ex = small.tile([P, PZ, H0], F16, tag="ex")
                            nc.scalar.activation(
                                out=ex[:, :bs], in_=lg[:, :bs], func=AF.Exp)
                            nc.vector.tensor_tensor(
                                out=zx[:, j0:j0 + bs, :hd].rearrange(
                                    "p j (h d) -> p j h d", h=h_),
                                in0=zps[:, :bs, :hd].rearrange(
                                    "p j (h d) -> p j h d", h=h_),
                                in1=ex[:, :bs].unsqueeze(3).to_broadcast(
                                    [P, bs, h_, D0]),
                                op=ALU.mult)
                            nc.vector.tensor_copy(
                                out=zx[:, j0:j0 + bs, hd:hd + h_],
                                in_=ex[:, :bs])
                    else:
                        gidx = sbuf.tile([P, G * 8], I16, tag="gidx")
                        nc.sync.dma_start(
                            out=gidx[:, :gsz * 8],
                            in_=ed["g"][:, g0 * 8:(g0 + gsz) * 8])
                        rowg = gb[:].rearrange("p (j w) -> p j w", w=P)
                        nc.gpsimd.dma_gather(
                            rowg[:, :gsz, :], z1el_tab[:], gidx[:, :gsz * 8],
                            ni, ni, P, single_packet=False)
                        ct = big.tile([WIN, G * P], F16, tag="ct")
                        nc.sync.dma_start(
                            out=ct[:, :ni],
                            in_=ed["ct"][:, g0 * P:(g0 + gsz) * P])
                        era = sbuf.tile([P, G], F32, tag="era")
                        for j0 in range(0, gsz, PZ):
                            bs = min(PZ, gsz - j0)
                            zps = psz.tile([P, PZ, RW0], F32, tag="ps_z")
                            for dj in range(bs):
                                j = j0 + dj
                                w = (g0 + j) // CPW
                                nc.tensor.matmul(
                                    out=zps[:, dj, :1],
                                    lhsT=ct[:, j * P:(j + 1) * P],
                                    rhs=erl1[:, w:w + 1],
                                    start=True, stop=True)
                            nc.vector.tensor_copy(
                                out=era[:, j0:j0 + bs],
                                in_=zps[:, :bs, 0])
                        lg = sbuf.tile([P, G, 1], F32, tag="lg1")
                        nc.vector.tensor_tensor(
                            out=lg[:, :gsz], in0=rowg[:, :gsz, hd:hd + 1],
                            in1=era[:, :gsz].unsqueeze(2), op=ALU.add)
                        nc.vector.scalar_tensor_tensor(
                            out=lg[:, :gsz], in0=lg[:, :gsz],
                            scalar=NEG_SLOPE, in1=lg[:, :gsz],
                            op0=ALU.mult, op1=ALU.max)
                        ex = sbuf.tile([P, G, 1], F16, tag="ex1")
                        nc.scalar.activation(
                            out=ex[:, :gsz], in_=lg[:, :gsz], func=AF.Exp)
                        nc.vector.tensor_tensor(
                            out=zx[:, :gsz, :hd], in0=rowg[:, :gsz, :hd],
                            in1=ex[:, :gsz].to_broadcast([P, gsz, hd]),
                            op=ALU.mult)
                        nc.vector.tensor_copy(
                            out=zx[:, :gsz, hd:hd + 1], in_=ex[:, :gsz])

                    for j in range(gsz):
                        chn = g0 + j
                        w = chn // CPW
                        first = chn % CPW == 0
                        last = chn % CPW == CPW - 1
                        if first:
                            psw_cur[0] = psw.tile(
                                [WIN, rw], F32, tag="ps_w",
                                name=f"pw_{name}_{w}")
                        nc.tensor.matmul(
                            out=psw_cur[0][:],
                            lhsT=cm[:, j * WIN:(j + 1) * WIN],
                            rhs=zx[:, j, :rw], start=first, stop=last)
                        if last:
                            pw = psw_cur[0]
                            sm = small.tile([WIN, h_], F32, tag="sm")
                            nc.vector.tensor_scalar(
                                out=sm[:], in0=pw[:, hd:hd + h_],
                                scalar1=SEG_EPS, scalar2=None, op0=ALU.max)
                            rs = small.tile([WIN, h_], F32, tag="rs")
                            nc.vector.reciprocal(out=rs[:], in_=sm[:])
                            if stg["t"] is None:
                                stg["t"] = stage.tile(
                                    [WIN, SB * hd], F32, tag="hstg",
                                    name=f"stg_{name}_{w}")
                            slot = w % SB
                            dstv = stg["t"][:, slot * hd:(slot + 1) * hd] \
                                .rearrange("d (h f) -> d h f", h=h_)
                            nc.vector.tensor_tensor(
                                out=dstv,
                                in0=pw[:, :hd].rearrange(
                                    "d (h f) -> d h f", h=h_),
                                in1=rs[:].unsqueeze(2).to_broadcast(
                                    [WIN, h_, hd // h_]),
                                op=ALU.mult)
                            if slot == SB - 1 or w == nw - 1:
                                flush_stage(w)

            # ---- Phase B: layer-0 edges ---------------------------------
            edge_phase("e0a", 0, h_stage_a)
            edge_phase("e0b", 0, h_stage_b)

            # ---- Phase C: h build + z1el shard + AllGather --------------
            n_ht = cdiv(cfg.R0S, P)
            for i in range(n_ht):
                p = min(P, cfg.R0S - i * P)
                at = sbuf.tile([P, HD0], F32, tag="ha")
                bt = sbuf.tile([P, HD0], F32, tag="hb")
                nc.sync.dma_start(out=at[:p], in_=h_stage_a[i * P:i * P + p, :])
                nc.sync.dma_start(out=bt[:p], in_=h_stage_b[i * P:i * P + p, :])
                nc.vector.tensor_add(out=at[:p], in0=at[:p], in1=bt[:p])
                hs = sbuf.tile([P, D0], F32, tag="hs")
                nc.vector.tensor_add(out=hs[:p], in0=at[:p, 0:D0],
                                     in1=at[:p, D0:2 * D0])
                for h in range(2, H0):
                    nc.vector.tensor_add(
                        out=hs[:p], in0=hs[:p],
                        in1=at[:p, h * D0:(h + 1) * D0])
                nc.vector.tensor_add(out=hs[:p], in0=hs[:p], in1=mb2_sb[:p])
                hr = sbuf.tile([P, D0], F16, tag="hr")
                nc.scalar.activation(out=hr[:p], in_=hs[:p], func=AF.Relu,
                                     scale=1.0 / H0)
                htp = psm.tile([D0, P], F16, tag="ps_m16")
                nc.tensor.transpose(out=htp[:, :p], in_=hr[:p],
                                    identity=ident16[:p, :p])
                hts = sbuf.tile([D0, P], F16, tag="hts")
                nc.vector.tensor_copy(out=hts[:, :p], in_=htp[:, :p])
                zp1 = psm.tile([P, RW1 + 1], F32, tag="ps_m")
                nc.tensor.matmul(out=zp1[:p], lhsT=hts[:, :p], rhs=W1e[:],
                                 start=True, stop=True)
                z1s = sbuf.tile([P, RW1 + 1], F16, tag="z1s")
                nc.vector.tensor_copy(out=z1s[:p], in_=zp1[:p])
                nc.sync.dma_start(out=z1el_shard[i * P:i * P + p, :],
                                  in_=z1s[:p])

            nc.gpsimd.collective_compute(
                "AllGather", ALU.bypass,
                replica_groups=[list(range(n_cores))],
                ins=[z1el_shard.opt()], outs=[z1el_small.opt()])

            # restride compact rows into 256B gather rows + zero row
            nc.sync.dma_start(out=z1el_tab[:cfg.NTAB, :RW1 + 1],
                              in_=z1el_small[:])
            nc.sync.dma_start(out=z1el_tab[cfg.ZROW:cfg.ZROW + 1, :RW1 + 1],
                              in_=zrow_sb[:])

            # er1 window values: gather dst-slot rows, extract er col into
            # erl1[s, w] (slot-partition, window-free) for the er matmuls
            e1idx = const.tile([P, NQ * 8], I16)
            nc.sync.dma_start(out=e1idx[:], in_=er1rows[:])
            e1g = const.tile([P, NQ, P], F16)
            nc.gpsimd.dma_gather(
                e1g[:], z1el_tab[:], e1idx[:], cfg.NER1, cfg.NER1, P,
                single_packet=False)
            erl1 = const.tile([WIN, NQ * CPW], F16)
            erl13 = erl1[:].rearrange("s (q c) -> s q c", c=CPW)
            for q2 in range(CPW):
                nc.sync.dma_start(
                    out=erl13[:, :, q2:q2 + 1],
                    in_=e1g[q2 * WIN:(q2 + 1) * WIN, :, RW1:RW1 + 1])

            # ---- Phase D: layer-1 edges ---------------------------------
            edge_phase("e1a", 1, o_stage_a, erl1=erl1)
            edge_phase("e1b", 1, o_stage_b, erl1=erl1)

            # ---- final combine ------------------------------------------
            n_ot = cdiv(cfg.R1S, P)
            for i in range(n_ot):
                p = min(P, cfg.R1S - i * P)
                oa = sbuf.tile([P, D1], F32, tag="oa")
                ob = sbuf.tile([P, D1], F32, tag="ob")
                nc.sync.dma_start(out=oa[:p], in_=o_stage_a[i * P:i * P + p, :])
                nc.sync.dma_start(out=ob[:p], in_=o_stage_b[i * P:i * P + p, :])
                nc.vector.tensor_add(out=oa[:p], in0=oa[:p], in1=ob[:p])
                nc.vector.tensor_add(out=oa[:p], in0=oa[:p], in1=bias1_sb[:p])
                nc.sync.dma_start(out=out_t[i * P:i * P + p, :], in_=oa[:p])

    nc.compile()
    return nc


# ----------------------------------------------------------------------------
# host driver
# ----------------------------------------------------------------------------

_CACHED = {}


def make_shared(cfg, inputs):
    return dict(
        W0=np.asarray(inputs["W0"], np.float32),
        Aler0=np.concatenate(
            [block_diag_attn(np.asarray(inputs["attn_l0"], np.float32)),
             block_diag_attn(np.asarray(inputs["attn_r0"], np.float32))],
            axis=1),
        b0r=np.asarray(inputs["b0"], np.float32).reshape(cfg.H0, cfg.D0),
        W1=np.asarray(inputs["W1"], np.float32),
        Aler1=np.concatenate(
            [block_diag_attn(np.asarray(inputs["attn_l1"], np.float32)),
             block_diag_attn(np.asarray(inputs["attn_r1"], np.float32))],
            axis=1),
        b1r=np.asarray(inputs["b1"], np.float32).reshape(1, cfg.D1),
        ones4=np.ones((cfg.H0, 1), np.float32),
        twos=np.full((1, P), 2.0, np.float32),
    )


def kernel(**inputs):
    dims = (8, 50000, 20000, 10000, 2500, 1250)
    cfg, per_core, perm1_list = prep_all(
        dims, inputs["feat"], inputs["src0a"], inputs["dst0a"],
        inputs["src0b"], inputs["dst0b"], inputs["src1a"], inputs["dst1a"],
        inputs["src1b"], inputs["dst1b"])

    key = (cfg.NW0, cfg.NW1)
    if key not in _CACHED:
        _CACHED[key] = build_program(cfg)
    nc = _CACHED[key]

    shared = make_shared(cfg, inputs)
    in_maps = []
    for c in range(cfg.n_cores):
        m = dict(shared)
        m.update(per_core[c])
        in_maps.append(m)

    res = run_bass_kernel_spmd(nc, in_maps, list(range(cfg.n_cores)))

    full = np.zeros((cfg.N2, cfg.D1), np.float32)
    for c in range(cfg.n_cores):
        o = res.results[c]["out"]
        perm1 = perm1_list[c]
        v = perm1 >= 0
        full[c * cfg.R1 + perm1[v]] = o[v]
    return full.reshape(cfg.N2, cfg.H1, cfg.D1).astype(np.float32)


# revision 24
# speedup vs baseline: 2.6173x; 1.0590x over previous
"""2-layer multi-edge-type GAT on Trainium2, 8-core SPMD (v3).

Key bottleneck on TRN2: gpsimd SWDGE descriptor generation runs at ~8-10 ns
per gathered row, so per-edge gathers cost ~1 ms/layer. This version
eliminates ALL layer-0 gathers: the edge list is known host-side, so the
host stages edge-ordered transposed feat blocks (pure data movement, no
flops) that the device reads with dense DMAs. Layer-1 sources reference the
device-computed h, so one batched int16 dma_gather per 64-chunk group
remains (the only per-edge descriptors in the kernel).

Structure per core (edges sharded by dst owner; dst nodes bin-packed into
32-slot windows jointly balanced over both edge types, <=512 edges/type per
window = exactly 4 chunks of 128):
  A) er0 window table: er = (feat @ W0)*attn_r for the permuted dst list,
     laid out [32 slots, NW0*H] for per-window matmul access.
  B) Layer-0 edges: per chunk, fp16 matmul feat_edges.T @ [W0|W0*attn_l]
     gives [z|el] in PSUM; a second small matmul cmpT.T @ er_win adds er;
     DVE: ex = exp(leakyrelu(el+er)) and rows = [ex*z | ex] fp16; one fp16
     one-hot window matmul accumulates [num|den] per window in PSUM;
     window flush normalizes (softmax shift dropped: logits O(1)).
  C) h = relu(mean_h(gat_a+gat_b)); z1el rows [z1|el1|er1] AllGathered
     compact, restrided to 256B rows for gathers.
  D) Layer-1 edges: batched dma_gather of [z1|el1] rows by src; er1 per
     edge via DVE multiply-reduce of the one-hot against window er values;
     same window aggregation; out = norm_a + norm_b + 2*b1.

Padding edges use zero feat blocks / a zero table row and all-zero one-hot
columns, so they contribute exactly nothing (ex stays finite, fp16-safe).
"""

import sys

import numpy as np

if "/opt/trn_rl_repo" not in sys.path:
    sys.path.insert(0, "/opt/trn_rl_repo")

import concourse.bacc as bacc
import concourse.bass as bass
import concourse.mybir as mybir
import concourse.tile as tile
from concourse.bass_utils import run_bass_kernel_spmd
from concourse.masks import make_identity

F32 = mybir.dt.float32
F16 = mybir.dt.float16
I16 = mybir.dt.int16
AF = mybir.ActivationFunctionType
ALU = mybir.AluOpType

P = 128
WIN = 32
CPW = 4            # chunks per window (512 edges)
CAP = WIN * 16     # 512-edge capacity per window per type
G = 64             # chunks per group
PZ = 3             # chunks per z-matmul psum batch
SB = 8             # windows per stage-flush batch
NEG_SLOPE = 0.2
SEG_EPS = 1e-9


def cdiv(a, b):
    return (a + b - 1) // b


# ----------------------------------------------------------------------------
# host-side (integer-only / data-movement-only) preprocessing
# ----------------------------------------------------------------------------

def pack_windows(dega, degb, nw):
    """Jointly bin-pack dsts into nw windows: <=32 slots, <=CAP edges of
    each type. Returns (win_of, slot_of) or None if infeasible."""
    r = len(dega)
    order = np.argsort(-(dega + degb), kind="stable")
    wca = np.zeros(nw, np.int64)
    wcb = np.zeros(nw, np.int64)
    wsl = np.zeros(nw, np.int64)
    win_of = np.full(r, -1, np.int64)
    slot_of = np.full(r, -1, np.int64)
    big = 1 << 40
    for d in order:
        da, db = dega[d], degb[d]
        cost = np.where(
            (wsl < WIN) & (wca + da <= CAP) & (wcb + db <= CAP),
            wca + wcb, big)
        w = int(np.argmin(cost))
        if cost[w] >= big:
            return None
        win_of[d] = w
        slot_of[d] = wsl[w]
        wsl[w] += 1
        wca[w] += da
        wcb[w] += db
    return win_of, slot_of


def edge_slots(src, d_loc, win_of, slot_of, nw):
    """Place edges into (chunk, partition) slots, window-major.
    Returns (srcs, cols): [CH, 128]; padding src=-1, col=-1."""
    ch = nw * CPW
    w = win_of[d_loc]
    s = slot_of[d_loc]
    order = np.lexsort((s, w))
    src_o, w_o, s_o = src[order], w[order], s[order]
    firsts = np.searchsorted(w_o, np.arange(nw), "left")
    pos = np.arange(len(w_o)) - firsts[w_o]
    assert pos.max(initial=0) < CAP
    chunk = w_o * CPW + pos // P
    part = pos % P
    srcs = np.full((ch, P), -1, np.int64)
    cols = np.full((ch, P), -1, np.int64)
    srcs[chunk, part] = src_o
    cols[chunk, part] = s_o
    return srcs, cols


def one_hots(cols):
    """cols [CH, 128] -> cm [128, CH*32] f16 and ct [32, CH*128] f16."""
    ch = cols.shape[0]
    rng32 = np.arange(WIN)
    oh = (cols[:, :, None] == rng32[None, None, :])        # [CH, 128, 32]
    cm = np.ascontiguousarray(
        oh.transpose(1, 0, 2).reshape(P, ch * WIN)).astype(np.float16)
    ct = np.ascontiguousarray(
        oh.transpose(2, 0, 1).reshape(WIN, ch * P)).astype(np.float16)
    return cm, ct


def wrap_groups(arr, group):
    """[CH, 128] int -> [128, CH*8] int16 wrapped per `group`-chunk group."""
    ch = arr.shape[0]
    out = np.zeros((128, ch * 8), np.int16)
    for g0 in range(0, ch, group):
        gsz = min(group, ch - g0)
        flat = arr[g0:g0 + gsz].reshape(-1)
        w16 = flat.reshape(-1, 16).T.astype(np.int16)
        out[:, g0 * 8:(g0 + gsz) * 8] = np.tile(w16, (8, 1))
    return out


def block_diag_attn(attn):
    h, d = attn.shape
    out = np.zeros((h * d, h), np.float32)
    for i in range(h):
        out[i * d:(i + 1) * d, i] = attn[i]
    return out


class Cfg:
    def __init__(self, nw0, nw1, n_cores=8, N0=50000, N1=20000,
                 N2=10000, F_in=64, H0=4, D0=32, H1=1, D1=32):
        self.n_cores = n_cores
        self.N0, self.N1, self.N2 = N0, N1, N2
        self.F_in, self.H0, self.D0, self.H1, self.D1 = F_in, H0, D0, H1, D1
        self.R0 = N1 // n_cores
        self.R1 = N2 // n_cores
        self.NW0, self.NW1 = nw0, nw1
        self.R0S = nw0 * WIN
        self.R1S = nw1 * WIN
        self.CH0 = nw0 * CPW
        self.CH1 = nw1 * CPW
        self.RW0 = H0 * D0 + H0   # 132
        self.RW1 = H1 * D1 + H1   # 33
        self.NTAB = n_cores * self.R0S   # z1el table rows; +1 zero row
        self.ZROW = self.NTAB
        self.NER1 = cdiv(self.R1S, P) * P


def prep_all(dims, feat, src0a, dst0a, src0b, dst0b, src1a, dst1a,
             src1b, dst1b):
    n_cores, N0, N1, N2, R0, R1 = dims
    feat = np.asarray(feat, np.float32)
    feat16 = feat.astype(np.float16)
    feat16z = np.vstack([feat16, np.zeros((1, feat.shape[1]), np.float16)])

    def split(src, dst, r):
        src, dst = np.asarray(src, np.int64), np.asarray(dst, np.int64)
        out = []
        for c in range(n_cores):
            m = (dst >= c * r) & (dst < (c + 1) * r)
            out.append((src[m], dst[m] - c * r))
        return out

    e0a, e0b = split(src0a, dst0a, R0), split(src0b, dst0b, R0)
    e1a, e1b = split(src1a, dst1a, R1), split(src1b, dst1b, R1)

    def pack_layer(ea, eb, r, nw_start):
        for nw in range(nw_start, nw_start + 6):
            packs = []
            ok = True
            for c in range(n_cores):
                dega = np.bincount(ea[c][1], minlength=r)
                degb = np.bincount(eb[c][1], minlength=r)
                pk = pack_windows(dega, degb, nw)
                if pk is None:
                    ok = False
                    break
                packs.append(pk)
            if ok:
                return nw, packs
        raise RuntimeError("window packing failed")

    nw0, packs0 = pack_layer(e0a, e0b, R0, 82)
    nw1, packs1 = pack_layer(e1a, e1b, R1, 42)
    cfg = Cfg(nw0, nw1, n_cores, N0, N1, N2)

    tabrow = np.zeros(N1, np.int64)
    for c in range(n_cores):
        win_of, slot_of = packs0[c]
        tabrow[c * R0 + np.arange(R0)] = c * cfg.R0S + win_of * WIN + slot_of

    per_core = []
    perm1_list = []
    for c in range(n_cores):
        m = {}
        win0, slot0 = packs0[c]
        win1, slot1 = packs1[c]
        for name, (src, d_loc) in (("e0a", e0a[c]), ("e0b", e0b[c])):
            srcs, cols = edge_slots(src, d_loc, win0, slot0, nw0)
            srcs[srcs < 0] = N0                    # zero feat row
            fe = feat16z[srcs.reshape(-1)].reshape(cfg.CH0, P, cfg.F_in)
            m[f"{name}_fe"] = np.ascontiguousarray(
                fe.transpose(2, 0, 1).reshape(cfg.F_in, cfg.CH0 * P))
            cm, ct = one_hots(cols)
            m[f"{name}_cm"] = cm
            m[f"{name}_ct"] = ct
        for name, (src, d_loc) in (("e1a", e1a[c]), ("e1b", e1b[c])):
            rows = tabrow[src]
            srcs, cols = edge_slots(rows, d_loc, win1, slot1, nw1)
            srcs[srcs < 0] = cfg.ZROW
            m[f"{name}_g"] = wrap_groups(srcs, G)
            cm, ct = one_hots(cols)
            m[f"{name}_cm"] = cm
            m[f"{name}_ct"] = ct
        # dst-permuted featT for er0 build (empty slots zero)
        ftd = np.zeros((cfg.F_in, cfg.R0S), np.float16)
        ftd[:, win0 * WIN + slot0] = feat16[c * R0:(c + 1) * R0].T
        m["featTdst"] = ftd
        # er1 window-value gather rows: slot i=(w*32+s) -> tabrow(dst)
        er1r = np.full(cfg.NER1, cfg.ZROW, np.int64)
        er1r[win1 * WIN + slot1] = tabrow[c * R1 + np.arange(R1)]
        m["er1rows"] = wrap_groups(er1r.reshape(-1, P), cdiv(cfg.NER1, P))
        per_core.append(m)
        perm1 = np.full(cfg.R1S, -1, np.int64)
        perm1[win1 * WIN + slot1] = np.arange(R1)
        perm1_list.append(perm1)
    return cfg, per_core, perm1_list


# ----------------------------------------------------------------------------
# device program
# ----------------------------------------------------------------------------

def build_program(cfg):
    n_cores = cfg.n_cores
    F_in, H0, D0, H1, D1 = cfg.F_in, cfg.H0, cfg.D0, cfg.H1, cfg.D1
    HD0, HD1 = H0 * D0, H1 * D1
    RW0, RW1 = cfg.RW0, cfg.RW1
    NQ = cfg.NER1 // P

    nc = bacc.Bacc("TRN2", target_bir_lowering=False, debug=False,
                   num_devices=n_cores)

    W0 = nc.dram_tensor("W0", [F_in, HD0], F32, kind="ExternalInput")
    Aler0 = nc.dram_tensor("Aler0", [HD0, 2 * H0], F32, kind="ExternalInput")
    b0r = nc.dram_tensor("b0r", [H0, D0], F32, kind="ExternalInput")
    W1 = nc.dram_tensor("W1", [D0, HD1], F32, kind="ExternalInput")
    Aler1 = nc.dram_tensor("Aler1", [HD1, 2 * H1], F32, kind="ExternalInput")
    b1r = nc.dram_tensor("b1r", [1, D1], F32, kind="ExternalInput")
    ones4 = nc.dram_tensor("ones4", [H0, 1], F32, kind="ExternalInput")
    twos = nc.dram_tensor("twos", [1, P], F32, kind="ExternalInput")
    featTdst = nc.dram_tensor("featTdst", [F_in, cfg.R0S], F16,
                              kind="ExternalInput")
    er1rows = nc.dram_tensor("er1rows", [P, NQ * 8], I16,
                             kind="ExternalInput")

    edge_in = {}
    for name in ("e0a", "e0b"):
        edge_in[name] = dict(
            fe=nc.dram_tensor(f"{name}_fe", [F_in, cfg.CH0 * P], F16,
                              kind="ExternalInput"),
            cm=nc.dram_tensor(f"{name}_cm", [P, cfg.CH0 * WIN], F16,
                              kind="ExternalInput"),
            ct=nc.dram_tensor(f"{name}_ct", [WIN, cfg.CH0 * P], F16,
                              kind="ExternalInput"),
            ch=cfg.CH0,
        )
    for name in ("e1a", "e1b"):
        edge_in[name] = dict(
            g=nc.dram_tensor(f"{name}_g", [P, cfg.CH1 * 8], I16,
                             kind="ExternalInput"),
            cm=nc.dram_tensor(f"{name}_cm", [P, cfg.CH1 * WIN], F16,
                              kind="ExternalInput"),
            ct=nc.dram_tensor(f"{name}_ct", [WIN, cfg.CH1 * P], F16,
                              kind="ExternalInput"),
            ch=cfg.CH1,
        )

    out_t = nc.dram_tensor("out", [cfg.R1S, D1], F32, kind="ExternalOutput")

    with tile.TileContext(nc) as tc:
        from contextlib import ExitStack
        with ExitStack() as ctx:
            const = ctx.enter_context(tc.tile_pool(name="const", bufs=1))
            sbuf = ctx.enter_context(tc.tile_pool(name="sbuf", bufs=3))
            big = ctx.enter_context(tc.tile_pool(name="big", bufs=2))
            stage = ctx.enter_context(tc.tile_pool(name="stage", bufs=2))
            small = ctx.enter_context(tc.tile_pool(name="small", bufs=3))
            psm = ctx.enter_context(tc.tile_pool(name="psm", bufs=1,
                                                 space="PSUM"))
            psz = ctx.enter_context(tc.tile_pool(name="psz", bufs=3,
                                                 space="PSUM"))
            psw = ctx.enter_context(tc.tile_pool(name="psw", bufs=3,
                                                 space="PSUM"))
            dram = ctx.enter_context(tc.tile_pool(name="dram", bufs=1,
                                                  space="DRAM"))

            # ---- internal DRAM ------------------------------------------
            erl0_dram = dram.tile([WIN, cfg.NW0 * H0], F16)
            h_stage_a = dram.tile([cfg.R0S, HD0], F32)
            h_stage_b = dram.tile([cfg.R0S, HD0], F32)
            z1el_shard = dram.tile([cfg.R0S, RW1 + 1], F16)
            z1el_small = dram.tile([cfg.NTAB, RW1 + 1], F16,
                                   addr_space="Shared")
            z1el_tab = dram.tile([cfg.NTAB + P, P], F16)
            o_stage_a = dram.tile([cfg.R1S, D1], F32)
            o_stage_b = dram.tile([cfg.R1S, D1], F32)

            # ---- constants ----------------------------------------------
            ident32 = const.tile([P, P], F32)
            make_identity(nc, ident32[:])
            ident16 = const.tile([P, P], F16)
            nc.vector.tensor_copy(out=ident16[:], in_=ident32[:])
            W0_sb = const.tile([F_in, HD0], F32)
            nc.sync.dma_start(out=W0_sb[:], in_=W0[:])
            Aler0_sb = const.tile([HD0, 2 * H0], F32)
            nc.sync.dma_start(out=Aler0_sb[:], in_=Aler0[:])
            b0_sb = const.tile([H0, D0], F32)
            nc.sync.dma_start(out=b0_sb[:], in_=b0r[:])
            W1_sb = const.tile([D0, HD1], F32)
            nc.sync.dma_start(out=W1_sb[:], in_=W1[:])
            Aler1_sb = const.tile([HD1, 2 * H1], F32)
            nc.sync.dma_start(out=Aler1_sb[:], in_=Aler1[:])
            b1_sb = const.tile([1, D1], F32)
            nc.sync.dma_start(out=b1_sb[:], in_=b1r[:])
            ones4_sb = const.tile([H0, 1], F32)
            nc.sync.dma_start(out=ones4_sb[:], in_=ones4[:])
            twos_sb = const.tile([1, P], F32)
            nc.sync.dma_start(out=twos_sb[:], in_=twos[:])

            pt = psm.tile([HD0, F_in], F32, tag="ps_m")
            nc.tensor.transpose(out=pt[:], in_=W0_sb[:],
                                identity=ident32[:F_in, :F_in])
            W0T_sb = const.tile([HD0, F_in], F32)
            nc.scalar.copy(out=W0T_sb[:], in_=pt[:])
            pe = psm.tile([F_in, 2 * H0], F32, tag="ps_m")
            nc.tensor.matmul(out=pe[:], lhsT=W0T_sb[:], rhs=Aler0_sb[:],
                             start=True, stop=True)
            W0el = const.tile([F_in, RW0], F16)
            nc.vector.tensor_copy(out=W0el[:, :HD0], in_=W0_sb[:])
            nc.vector.tensor_copy(out=W0el[:, HD0:], in_=pe[:, :H0])
            W0r = const.tile([F_in, H0], F16)
            nc.vector.tensor_copy(out=W0r[:], in_=pe[:, H0:])

            pt1 = psm.tile([HD1, D0], F32, tag="ps_m")
            nc.tensor.transpose(out=pt1[:], in_=W1_sb[:],
                                identity=ident32[:D0, :D0])
            W1T_sb = const.tile([HD1, D0], F32)
            nc.scalar.copy(out=W1T_sb[:], in_=pt1[:])
            pe1 = psm.tile([D0, 2 * H1], F32, tag="ps_m")
            nc.tensor.matmul(out=pe1[:], lhsT=W1T_sb[:], rhs=Aler1_sb[:],
                             start=True, stop=True)
            W1e = const.tile([D0, RW1 + 1], F16)
            nc.vector.tensor_copy(out=W1e[:, :HD1], in_=W1_sb[:])
            nc.vector.tensor_copy(out=W1e[:, HD1:], in_=pe1[:])

            ps_s = psm.tile([1, D0], F32, tag="ps_m")
            nc.tensor.matmul(out=ps_s[:], lhsT=ones4_sb[:], rhs=b0_sb[:],
                             start=True, stop=True)
            sb0_sb = const.tile([1, D0], F32)
            nc.scalar.copy(out=sb0_sb[:], in_=ps_s[:])
            ps_mb = psm.tile([P, D0], F32, tag="ps_m")
            nc.tensor.matmul(out=ps_mb[:], lhsT=twos_sb[:], rhs=sb0_sb[:],
                             start=True, stop=True)
            mb2_sb = const.tile([P, D0], F32)
            nc.scalar.copy(out=mb2_sb[:], in_=ps_mb[:])
            ps_b1 = psm.tile([P, D1], F32, tag="ps_m")
            nc.tensor.matmul(out=ps_b1[:], lhsT=twos_sb[:], rhs=b1_sb[:],
                             start=True, stop=True)
            bias1_sb = const.tile([P, D1], F32)
            nc.scalar.copy(out=bias1_sb[:], in_=ps_b1[:])

            zrow_sb = const.tile([1, RW1 + 1], F16)
            nc.vector.memset(zrow_sb[:], 0.0)

            # ---- Phase A': er0 window table -----------------------------
            n_dt = cdiv(cfg.R0S, P)
            for t in range(n_dt):
                p = min(P, cfg.R0S - t * P)
                q = p // WIN
                ftd = sbuf.tile([F_in, P], F16, tag="ftd")
                nc.sync.dma_start(out=ftd[:, :p],
                                  in_=featTdst[:, t * P:t * P + p])
                pse = psm.tile([P, H0], F32, tag="ps_m")
                nc.tensor.matmul(out=pse[:p], lhsT=ftd[:, :p], rhs=W0r[:],
                                 start=True, stop=True)
                st = sbuf.tile([P, H0], F16, tag="erst")
                nc.vector.tensor_copy(out=st[:p], in_=pse[:p])
                for qq in range(q):
                    nc.sync.dma_start(
                        out=erl0_dram[:, (4 * t + qq) * H0:
                                      (4 * t + qq + 1) * H0],
                        in_=st[qq * WIN:(qq + 1) * WIN, :])
            erl0 = const.tile([WIN, cfg.NW0 * H0], F16)
            nc.sync.dma_start(out=erl0[:], in_=erl0_dram[:])

            # ---- edge aggregation ---------------------------------------
            psw_cur = [None]

            def edge_phase(name, layer, stage_dram, erl1=None):
                ed = edge_in[name]
                ch = ed["ch"]
                h_, rw = (H0, RW0) if layer == 0 else (H1, RW1)
                hd = h_ * (D0 if layer == 0 else D1)
                nw = cfg.NW0 if layer == 0 else cfg.NW1
                stg = {"t": None}

                def flush_stage(w_hi):
                    w_lo = (w_hi // SB) * SB
                    k = w_hi - w_lo + 1
                    nc.sync.dma_start(
                        out=stage_dram[w_lo * WIN:(w_hi + 1) * WIN, :]
                        .rearrange("(j d) f -> d j f", d=WIN),
                        in_=stg["t"][:, :k * hd].rearrange(
                            "d (j f) -> d j f", f=hd))
                    stg["t"] = None

                for g0 in range(0, ch, G):
                    gsz = min(G, ch - g0)
                    ni = gsz * P
                    cm = big.tile([P, G * WIN], F16, tag="cm")
                    nc.sync.dma_start(
                        out=cm[:, :gsz * WIN],
                        in_=ed["cm"][:, g0 * WIN:(g0 + gsz) * WIN])
                    zx = big.tile([P, G, RW0], F16, tag="zx")
                    gb = big.tile([P, G * P], F16, tag="gbig")

                    if layer == 0:
                        fe = gb
                        nc.sync.dma_start(
                            out=fe[:F_in, :ni],
                            in_=ed["fe"][:, g0 * P:(g0 + gsz) * P])
                        ct = big.tile([WIN, G * P], F16, tag="ct")
                        nc.sync.dma_start(
                            out=ct[:, :ni],
                            in_=ed["ct"][:, g0 * P:(g0 + gsz) * P])
                        for j0 in range(0, gsz, PZ):
                            bs = min(PZ, gsz - j0)
                            zps = psz.tile([P, PZ, RW0], F32, tag="ps_z")
                            for dj in range(bs):
                                j = j0 + dj
                                nc.tensor.matmul(
                                    out=zps[:, dj, :],
                                    lhsT=fe[:F_in, j * P:(j + 1) * P],
                                    rhs=W0el[:], start=True, stop=True)
                                w = (g0 + j) // CPW
                                # accumulate er onto the el columns
                                nc.tensor.matmul(
                                    out=zps[:, dj, HD0:RW0],
                                    lhsT=ct[:, j * P:(j + 1) * P],
                                    rhs=erl0[:, w * H0:(w + 1) * H0],
                                    start=False, stop=True)
                            lg = small.tile([P, PZ, H0], F32, tag="lg")
                            nc.scalar.activation(
                                out=lg[:, :bs], in_=zps[:, :bs, HD0:RW0],
                                func=AF.Prelu, alpha=NEG_SLOPE)
                            nc.scalar.activation(
                                out=zx[:, j0:j0 + bs, hd:hd + h_],
                                in_=lg[:, :bs], func=AF.Exp)
                            nc.vector.tensor_tensor(
                                out=zx[:, j0:j0 + bs, :hd].rearrange(
                                    "p j (h d) -> p j h d", h=h_),
                                in0=zps[:, :bs, :hd].rearrange(
                                    "p j (h d) -> p j h d", h=h_),
                                in1=zx[:, j0:j0 + bs, hd:hd + h_]
                                .unsqueeze(3).to_broadcast([P, bs, h_, D0]),
                                op=ALU.mult)
                    else:
                        gidx = sbuf.tile([P, G * 8], I16, tag="gidx")
                        nc.sync.dma_start(
                            out=gidx[:, :gsz * 8],
                            in_=ed["g"][:, g0 * 8:(g0 + gsz) * 8])
                        rowg = gb[:].rearrange("p (j w) -> p j w", w=P)
                        nc.gpsimd.dma_gather(
                            rowg[:, :gsz, :], z1el_tab[:], gidx[:, :gsz * 8],
                            ni, ni, P, single_packet=False)
                        ct = big.tile([WIN, G * P], F16, tag="ct")
                        nc.sync.dma_start(
                            out=ct[:, :ni],
                            in_=ed["ct"][:, g0 * P:(g0 + gsz) * P])
                        era = sbuf.tile([P, G], F32, tag="era")
                        for j0 in range(0, gsz, PZ):
                            bs = min(PZ, gsz - j0)
                            zps = psz.tile([P, PZ, RW0], F32, tag="ps_z")
                            for dj in range(bs):
                                j = j0 + dj
                                w = (g0 + j) // CPW
                                nc.tensor.matmul(
                                    out=zps[:, dj, :1],
                                    lhsT=ct[:, j * P:(j + 1) * P],
                                    rhs=erl1[:, w:w + 1],
                                    start=True, stop=True)
                            nc.vector.tensor_copy(
                                out=era[:, j0:j0 + bs],
                                in_=zps[:, :bs, 0])
                        lg = sbuf.tile([P, G, 1], F32, tag="lg1")
                        nc.vector.tensor_tensor(
                            out=lg[:, :gsz], in0=rowg[:, :gsz, hd:hd + 1],
                            in1=era[:, :gsz].unsqueeze(2), op=ALU.add)
                        nc.scalar.activation(
                            out=lg[:, :gsz], in_=lg[:, :gsz],
                            func=AF.Prelu, alpha=NEG_SLOPE)
                        nc.scalar.activation(
                            out=zx[:, :gsz, hd:hd + 1], in_=lg[:, :gsz],
                            func=AF.Exp)
                        nc.vector.tensor_tensor(
                            out=zx[:, :gsz, :hd], in0=rowg[:, :gsz, :hd],
                            in1=zx[:, :gsz, hd:hd + 1]
                            .to_broadcast([P, gsz, hd]),
                            op=ALU.mult)

                    for j in range(gsz):
                        chn = g0 + j
                        w = chn // CPW
                        first = chn % CPW == 0
                        last = chn % CPW == CPW - 1
                        if first:
                            psw_cur[0] = psw.tile(
                                [WIN, rw], F32, tag="ps_w",
                                name=f"pw_{name}_{w}")
                        nc.tensor.matmul(
                            out=psw_cur[0][:],
                            lhsT=cm[:, j * WIN:(j + 1) * WIN],
                            rhs=zx[:, j, :rw], start=first, stop=last)
                        if last:
                            pw = psw_cur[0]
                            sm = small.tile([WIN, h_], F32, tag="sm")
                            nc.vector.tensor_scalar(
                                out=sm[:], in0=pw[:, hd:hd + h_],
                                scalar1=SEG_EPS, scalar2=None, op0=ALU.max)
                            rs = small.tile([WIN, h_], F32, tag="rs")
                            nc.vector.reciprocal(out=rs[:], in_=sm[:])
                            if stg["t"] is None:
                                stg["t"] = stage.tile(
                                    [WIN, SB * hd], F32, tag="hstg",
                                    name=f"stg_{name}_{w}")
                            slot = w % SB
                            dstv = stg["t"][:, slot * hd:(slot + 1) * hd] \
                                .rearrange("d (h f) -> d h f", h=h_)
                            nc.vector.tensor_tensor(
                                out=dstv,
                                in0=pw[:, :hd].rearrange(
                                    "d (h f) -> d h f", h=h_),
                                in1=rs[:].unsqueeze(2).to_broadcast(
                                    [WIN, h_, hd // h_]),
                                op=ALU.mult)
                            if slot == SB - 1 or w == nw - 1:
                                flush_stage(w)

            # ---- Phase B: layer-0 edges ---------------------------------
            edge_phase("e0a", 0, h_stage_a)
            edge_phase("e0b", 0, h_stage_b)

            # ---- Phase C: h build + z1el shard + AllGather --------------
            n_ht = cdiv(cfg.R0S, P)
            for i in range(n_ht):
                p = min(P, cfg.R0S - i * P)
                at = sbuf.tile([P, HD0], F32, tag="ha")
                bt = sbuf.tile([P, HD0], F32, tag="hb")
                nc.sync.dma_start(out=at[:p], in_=h_stage_a[i * P:i * P + p, :])
                nc.sync.dma_start(out=bt[:p], in_=h_stage_b[i * P:i * P + p, :])
                nc.vector.tensor_add(out=at[:p], in0=at[:p], in1=bt[:p])
                hs = sbuf.tile([P, D0], F32, tag="hs")
                nc.vector.tensor_add(out=hs[:p], in0=at[:p, 0:D0],
                                     in1=at[:p, D0:2 * D0])
                for h in range(2, H0):
                    nc.vector.tensor_add(
                        out=hs[:p], in0=hs[:p],
                        in1=at[:p, h * D0:(h + 1) * D0])
                nc.vector.tensor_add(out=hs[:p], in0=hs[:p], in1=mb2_sb[:p])
                hr = sbuf.tile([P, D0], F16, tag="hr")
                nc.scalar.activation(out=hr[:p], in_=hs[:p], func=AF.Relu,
                                     scale=1.0 / H0)
                htp = psm.tile([D0, P], F16, tag="ps_m16")
                nc.tensor.transpose(out=htp[:, :p], in_=hr[:p],
                                    identity=ident16[:p, :p])
                hts = sbuf.tile([D0, P], F16, tag="hts")
                nc.vector.tensor_copy(out=hts[:, :p], in_=htp[:, :p])
                zp1 = psm.tile([P, RW1 + 1], F32, tag="ps_m")
                nc.tensor.matmul(out=zp1[:p], lhsT=hts[:, :p], rhs=W1e[:],
                                 start=True, stop=True)
                z1s = sbuf.tile([P, RW1 + 1], F16, tag="z1s")
                nc.vector.tensor_copy(out=z1s[:p], in_=zp1[:p])
                nc.sync.dma_start(out=z1el_shard[i * P:i * P + p, :],
                                  in_=z1s[:p])

            nc.gpsimd.collective_compute(
                "AllGather", ALU.bypass,
                replica_groups=[list(range(n_cores))],
                ins=[z1el_shard.opt()], outs=[z1el_small.opt()])

            # restride compact rows into 256B gather rows + zero row
            nc.sync.dma_start(out=z1el_tab[:cfg.NTAB, :RW1 + 1],
                              in_=z1el_small[:])
            nc.sync.dma_start(out=z1el_tab[cfg.ZROW:cfg.ZROW + 1, :RW1 + 1],
                              in_=zrow_sb[:])

            # er1 window values: gather dst-slot rows, extract er col into
            # erl1[s, w] (slot-partition, window-free) for the er matmuls
            e1idx = const.tile([P, NQ * 8], I16)
            nc.sync.dma_start(out=e1idx[:], in_=er1rows[:])
            e1g = const.tile([P, NQ, P], F16)
            nc.gpsimd.dma_gather(
                e1g[:], z1el_tab[:], e1idx[:], cfg.NER1, cfg.NER1, P,
                single_packet=False)
            erl1 = const.tile([WIN, NQ * CPW], F16)
            erl13 = erl1[:].rearrange("s (q c) -> s q c", c=CPW)
            for q2 in range(CPW):
                nc.sync.dma_start(
                    out=erl13[:, :, q2:q2 + 1],
                    in_=e1g[q2 * WIN:(q2 + 1) * WIN, :, RW1:RW1 + 1])

            # ---- Phase D: layer-1 edges ---------------------------------
            edge_phase("e1a", 1, o_stage_a, erl1=erl1)
            edge_phase("e1b", 1, o_stage_b, erl1=erl1)

            # ---- final combine ------------------------------------------
            n_ot = cdiv(cfg.R1S, P)
            for i in range(n_ot):
                p = min(P, cfg.R1S - i * P)
                oa = sbuf.tile([P, D1], F32, tag="oa")
                ob = sbuf.tile([P, D1], F32, tag="ob")
                nc.sync.dma_start(out=oa[:p], in_=o_stage_a[i * P:i * P + p, :])
                nc.sync.dma_start(out=ob[:p], in_=o_stage_b[i * P:i * P + p, :])
                nc.vector.tensor_add(out=oa[:p], in0=oa[:p], in1=ob[:p])
                nc.vector.tensor_add(out=oa[:p], in0=oa[:p], in1=bias1_sb[:p])
                nc.sync.dma_start(out=out_t[i * P:i * P + p, :], in_=oa[:p])

    nc.compile()
    return nc


# ----------------------------------------------------------------------------
# host driver
# ----------------------------------------------------------------------------

_CACHED = {}


def make_shared(cfg, inputs):
    return dict(
        W0=np.asarray(inputs["W0"], np.float32),
        Aler0=np.concatenate(
            [block_diag_attn(np.asarray(inputs["attn_l0"], np.float32)),
             block_diag_attn(np.asarray(inputs["attn_r0"], np.float32))],
            axis=1),
        b0r=np.asarray(inputs["b0"], np.float32).reshape(cfg.H0, cfg.D0),
        W1=np.asarray(inputs["W1"], np.float32),
        Aler1=np.concatenate(
            [block_diag_attn(np.asarray(inputs["attn_l1"], np.float32)),
             block_diag_attn(np.asarray(inputs["attn_r1"], np.float32))],
            axis=1),
        b1r=np.asarray(inputs["b1"], np.float32).reshape(1, cfg.D1),
        ones4=np.ones((cfg.H0, 1), np.float32),
        twos=np.full((1, P), 2.0, np.float32),
    )


def kernel(**inputs):
    dims = (8, 50000, 20000, 10000, 2500, 1250)
    cfg, per_core, perm1_list = prep_all(
        dims, inputs["feat"], inputs["src0a"], inputs["dst0a"],
        inputs["src0b"], inputs["dst0b"], inputs["src1a"], inputs["dst1a"],
        inputs["src1b"], inputs["dst1b"])

    key = (cfg.NW0, cfg.NW1)
    if key not in _CACHED:
        _CACHED[key] = build_program(cfg)
    nc = _CACHED[key]

    shared = make_shared(cfg, inputs)
    in_maps = []
    for c in range(cfg.n_cores):
        m = dict(shared)
        m.update(per_core[c])
        in_maps.append(m)

    res = run_bass_kernel_spmd(nc, in_maps, list(range(cfg.n_cores)))

    full = np.zeros((cfg.N2, cfg.D1), np.float32)
    for c in range(cfg.n_cores):
        o = res.results[c]["out"]
        perm1 = perm1_list[c]
        v = perm1 >= 0
        full[c * cfg.R1 + perm1[v]] = o[v]
    return full.reshape(cfg.N2, cfg.H1, cfg.D1).astype(np.float32)


# revision 27
# speedup vs baseline: 2.7675x; 1.0574x over previous
"""2-layer multi-edge-type GAT on Trainium2, 8-core SPMD (v3).

Key bottleneck on TRN2: gpsimd SWDGE descriptor generation runs at ~8-10 ns
per gathered row, so per-edge gathers cost ~1 ms/layer. This version
eliminates ALL layer-0 gathers: the edge list is known host-side, so the
host stages edge-ordered transposed feat blocks (pure data movement, no
flops) that the device reads with dense DMAs. Layer-1 sources reference the
device-computed h, so one batched int16 dma_gather per 64-chunk group
remains (the only per-edge descriptors in the kernel).

Structure per core (edges sharded by dst owner; dst nodes bin-packed into
32-slot windows jointly balanced over both edge types, <=512 edges/type per
window = exactly 4 chunks of 128):
  A) er0 window table: er = (feat @ W0)*attn_r for the permuted dst list,
     laid out [32 slots, NW0*H] for per-window matmul access.
  B) Layer-0 edges: per chunk, fp16 matmul feat_edges.T @ [W0|W0*attn_l]
     gives [z|el] in PSUM; a second small matmul cmpT.T @ er_win adds er;
     DVE: ex = exp(leakyrelu(el+er)) and rows = [ex*z | ex] fp16; one fp16
     one-hot window matmul accumulates [num|den] per window in PSUM;
     window flush normalizes (softmax shift dropped: logits O(1)).
  C) h = relu(mean_h(gat_a+gat_b)); z1el rows [z1|el1|er1] AllGathered
     compact, restrided to 256B rows for gathers.
  D) Layer-1 edges: batched dma_gather of [z1|el1] rows by src; er1 per
     edge via DVE multiply-reduce of the one-hot against window er values;
     same window aggregation; out = norm_a + norm_b + 2*b1.

Padding edges use zero feat blocks / a zero table row and all-zero one-hot
columns, so they contribute exactly nothing (ex stays finite, fp16-safe).
"""

import sys

import numpy as np

if "/opt/trn_rl_repo" not in sys.path:
    sys.path.insert(0, "/opt/trn_rl_repo")

import concourse.bacc as bacc
import concourse.bass as bass
import concourse.mybir as mybir
import concourse.tile as tile
from concourse.bass_utils import run_bass_kernel_spmd
from concourse.masks import make_identity

F32 = mybir.dt.float32
F16 = mybir.dt.float16
I16 = mybir.dt.int16
AF = mybir.ActivationFunctionType
ALU = mybir.AluOpType

P = 128
WIN = 32
CPW = 4            # chunks per window (512 edges)
CAP = WIN * 16     # 512-edge capacity per window per type
G = 64             # chunks per group (layer 0)
G1 = 28            # chunks per gather group (layer 1)
PZ = 3             # chunks per z-matmul psum batch
SB = 8             # windows per stage-flush batch
NEG_SLOPE = 0.2
SEG_EPS = 1e-9


def cdiv(a, b):
    return (a + b - 1) // b


# ----------------------------------------------------------------------------
# host-side (integer-only / data-movement-only) preprocessing
# ----------------------------------------------------------------------------

def pack_windows(dega, degb, nw):
    """Jointly bin-pack dsts into nw windows: <=32 slots, <=CAP edges of
    each type. Returns (win_of, slot_of) or None if infeasible."""
    r = len(dega)
    order = np.argsort(-(dega + degb), kind="stable")
    wca = np.zeros(nw, np.int64)
    wcb = np.zeros(nw, np.int64)
    wsl = np.zeros(nw, np.int64)
    win_of = np.full(r, -1, np.int64)
    slot_of = np.full(r, -1, np.int64)
    big = 1 << 40
    for d in order:
        da, db = dega[d], degb[d]
        cost = np.where(
            (wsl < WIN) & (wca + da <= CAP) & (wcb + db <= CAP),
            wca + wcb, big)
        w = int(np.argmin(cost))
        if cost[w] >= big:
            return None
        win_of[d] = w
        slot_of[d] = wsl[w]
        wsl[w] += 1
        wca[w] += da
        wcb[w] += db
    return win_of, slot_of


def edge_slots(src, d_loc, win_of, slot_of, nw):
    """Place edges into (chunk, partition) slots, window-major.
    Returns (srcs, cols): [CH, 128]; padding src=-1, col=-1."""
    ch = nw * CPW
    w = win_of[d_loc]
    s = slot_of[d_loc]
    order = np.lexsort((s, w))
    src_o, w_o, s_o = src[order], w[order], s[order]
    firsts = np.searchsorted(w_o, np.arange(nw), "left")
    pos = np.arange(len(w_o)) - firsts[w_o]
    assert pos.max(initial=0) < CAP
    chunk = w_o * CPW + pos // P
    part = pos % P
    srcs = np.full((ch, P), -1, np.int64)
    cols = np.full((ch, P), -1, np.int64)
    srcs[chunk, part] = src_o
    cols[chunk, part] = s_o
    return srcs, cols


def one_hots(cols):
    """cols [CH, 128] -> cm [128, CH*32] f16 and ct [32, CH*128] f16."""
    ch = cols.shape[0]
    rng32 = np.arange(WIN)
    oh = (cols[:, :, None] == rng32[None, None, :])        # [CH, 128, 32]
    cm = np.ascontiguousarray(
        oh.transpose(1, 0, 2).reshape(P, ch * WIN)).astype(np.float16)
    ct = np.ascontiguousarray(
        oh.transpose(2, 0, 1).reshape(WIN, ch * P)).astype(np.float16)
    return cm, ct


def wrap_groups(arr, group):
    """[CH, 128] int -> [128, CH*8] int16 wrapped per `group`-chunk group."""
    ch = arr.shape[0]
    out = np.zeros((128, ch * 8), np.int16)
    for g0 in range(0, ch, group):
        gsz = min(group, ch - g0)
        flat = arr[g0:g0 + gsz].reshape(-1)
        w16 = flat.reshape(-1, 16).T.astype(np.int16)
        out[:, g0 * 8:(g0 + gsz) * 8] = np.tile(w16, (8, 1))
    return out


def block_diag_attn(attn):
    h, d = attn.shape
    out = np.zeros((h * d, h), np.float32)
    for i in range(h):
        out[i * d:(i + 1) * d, i] = attn[i]
    return out


class Cfg:
    def __init__(self, nw0, nw1, n_cores=8, N0=50000, N1=20000,
                 N2=10000, F_in=64, H0=4, D0=32, H1=1, D1=32):
        self.n_cores = n_cores
        self.N0, self.N1, self.N2 = N0, N1, N2
        self.F_in, self.H0, self.D0, self.H1, self.D1 = F_in, H0, D0, H1, D1
        self.R0 = N1 // n_cores
        self.R1 = N2 // n_cores
        self.NW0, self.NW1 = nw0, nw1
        self.R0S = nw0 * WIN
        self.R1S = nw1 * WIN
        self.CH0 = nw0 * CPW
        self.CH1 = nw1 * CPW
        self.RW0 = H0 * D0 + H0   # 132
        self.RW1 = H1 * D1 + H1   # 33
        self.NTAB = n_cores * self.R0S   # z1el table rows; +1 zero row
        self.ZROW = self.NTAB
        self.NER1 = cdiv(self.R1S, P) * P


def prep_all(dims, feat, src0a, dst0a, src0b, dst0b, src1a, dst1a,
             src1b, dst1b):
    n_cores, N0, N1, N2, R0, R1 = dims
    feat = np.asarray(feat, np.float32)
    feat16 = feat.astype(np.float16)
    feat16z = np.vstack([feat16, np.zeros((1, feat.shape[1]), np.float16)])

    def split(src, dst, r):
        src, dst = np.asarray(src, np.int64), np.asarray(dst, np.int64)
        out = []
        for c in range(n_cores):
            m = (dst >= c * r) & (dst < (c + 1) * r)
            out.append((src[m], dst[m] - c * r))
        return out

    e0a, e0b = split(src0a, dst0a, R0), split(src0b, dst0b, R0)
    e1a, e1b = split(src1a, dst1a, R1), split(src1b, dst1b, R1)

    def pack_layer(ea, eb, r, nw_start):
        for nw in range(nw_start, nw_start + 6):
            packs = []
            ok = True
            for c in range(n_cores):
                dega = np.bincount(ea[c][1], minlength=r)
                degb = np.bincount(eb[c][1], minlength=r)
                pk = pack_windows(dega, degb, nw)
                if pk is None:
                    ok = False
                    break
                packs.append(pk)
            if ok:
                return nw, packs
        raise RuntimeError("window packing failed")

    nw0, packs0 = pack_layer(e0a, e0b, R0, 82)
    nw1, packs1 = pack_layer(e1a, e1b, R1, 42)
    cfg = Cfg(nw0, nw1, n_cores, N0, N1, N2)

    tabrow = np.zeros(N1, np.int64)
    for c in range(n_cores):
        win_of, slot_of = packs0[c]
        tabrow[c * R0 + np.arange(R0)] = c * cfg.R0S + win_of * WIN + slot_of

    per_core = []
    perm1_list = []
    for c in range(n_cores):
        m = {}
        win0, slot0 = packs0[c]
        win1, slot1 = packs1[c]
        for name, (src, d_loc) in (("e0a", e0a[c]), ("e0b", e0b[c])):
            srcs, cols = edge_slots(src, d_loc, win0, slot0, nw0)
            srcs[srcs < 0] = N0                    # zero feat row
            fe = feat16z[srcs.reshape(-1)].reshape(cfg.CH0, P, cfg.F_in)
            m[f"{name}_fe"] = np.ascontiguousarray(
                fe.transpose(2, 0, 1).reshape(cfg.F_in, cfg.CH0 * P))
            cm, ct = one_hots(cols)
            m[f"{name}_cm"] = cm
            m[f"{name}_ct"] = ct
        for name, (src, d_loc) in (("e1a", e1a[c]), ("e1b", e1b[c])):
            rows = tabrow[src]
            srcs, cols = edge_slots(rows, d_loc, win1, slot1, nw1)
            srcs[srcs < 0] = cfg.ZROW
            m[f"{name}_g"] = wrap_groups(srcs, G1)
            cm, ct = one_hots(cols)
            m[f"{name}_cm"] = cm
            m[f"{name}_ct"] = ct
        # dst-permuted featT for er0 build (empty slots zero)
        ftd = np.zeros((cfg.F_in, cfg.R0S), np.float16)
        ftd[:, win0 * WIN + slot0] = feat16[c * R0:(c + 1) * R0].T
        m["featTdst"] = ftd
        # er1 window-value gather rows: slot i=(w*32+s) -> tabrow(dst)
        er1r = np.full(cfg.NER1, cfg.ZROW, np.int64)
        er1r[win1 * WIN + slot1] = tabrow[c * R1 + np.arange(R1)]
        m["er1rows"] = wrap_groups(er1r.reshape(-1, P), cdiv(cfg.NER1, P))
        per_core.append(m)
        perm1 = np.full(cfg.R1S, -1, np.int64)
        perm1[win1 * WIN + slot1] = np.arange(R1)
        perm1_list.append(perm1)
    return cfg, per_core, perm1_list


# ----------------------------------------------------------------------------
# device program
# ----------------------------------------------------------------------------

def build_program(cfg):
    n_cores = cfg.n_cores
    F_in, H0, D0, H1, D1 = cfg.F_in, cfg.H0, cfg.D0, cfg.H1, cfg.D1
    HD0, HD1 = H0 * D0, H1 * D1
    RW0, RW1 = cfg.RW0, cfg.RW1
    NQ = cfg.NER1 // P

    nc = bacc.Bacc("TRN2", target_bir_lowering=False, debug=False,
                   num_devices=n_cores)

    W0 = nc.dram_tensor("W0", [F_in, HD0], F32, kind="ExternalInput")
    Aler0 = nc.dram_tensor("Aler0", [HD0, 2 * H0], F32, kind="ExternalInput")
    b0r = nc.dram_tensor("b0r", [H0, D0], F32, kind="ExternalInput")
    W1 = nc.dram_tensor("W1", [D0, HD1], F32, kind="ExternalInput")
    Aler1 = nc.dram_tensor("Aler1", [HD1, 2 * H1], F32, kind="ExternalInput")
    b1r = nc.dram_tensor("b1r", [1, D1], F32, kind="ExternalInput")
    ones4 = nc.dram_tensor("ones4", [H0, 1], F32, kind="ExternalInput")
    twos = nc.dram_tensor("twos", [1, P], F32, kind="ExternalInput")
    featTdst = nc.dram_tensor("featTdst", [F_in, cfg.R0S], F16,
                              kind="ExternalInput")
    er1rows = nc.dram_tensor("er1rows", [P, NQ * 8], I16,
                             kind="ExternalInput")

    edge_in = {}
    for name in ("e0a", "e0b"):
        edge_in[name] = dict(
            fe=nc.dram_tensor(f"{name}_fe", [F_in, cfg.CH0 * P], F16,
                              kind="ExternalInput"),
            cm=nc.dram_tensor(f"{name}_cm", [P, cfg.CH0 * WIN], F16,
                              kind="ExternalInput"),
            ct=nc.dram_tensor(f"{name}_ct", [WIN, cfg.CH0 * P], F16,
                              kind="ExternalInput"),
            ch=cfg.CH0,
        )
    for name in ("e1a", "e1b"):
        edge_in[name] = dict(
            g=nc.dram_tensor(f"{name}_g", [P, cfg.CH1 * 8], I16,
                             kind="ExternalInput"),
            cm=nc.dram_tensor(f"{name}_cm", [P, cfg.CH1 * WIN], F16,
                              kind="ExternalInput"),
            ct=nc.dram_tensor(f"{name}_ct", [WIN, cfg.CH1 * P], F16,
                              kind="ExternalInput"),
            ch=cfg.CH1,
        )

    out_t = nc.dram_tensor("out", [cfg.R1S, D1], F32, kind="ExternalOutput")

    with tile.TileContext(nc) as tc:
        from contextlib import ExitStack
        with ExitStack() as ctx:
            const = ctx.enter_context(tc.tile_pool(name="const", bufs=1))
            sbuf = ctx.enter_context(tc.tile_pool(name="sbuf", bufs=3))
            big = ctx.enter_context(tc.tile_pool(name="big", bufs=2))
            stage = ctx.enter_context(tc.tile_pool(name="stage", bufs=2))
            small = ctx.enter_context(tc.tile_pool(name="small", bufs=3))
            psm = ctx.enter_context(tc.tile_pool(name="psm", bufs=1,
                                                 space="PSUM"))
            psz = ctx.enter_context(tc.tile_pool(name="psz", bufs=3,
                                                 space="PSUM"))
            psw = ctx.enter_context(tc.tile_pool(name="psw", bufs=3,
                                                 space="PSUM"))
            dram = ctx.enter_context(tc.tile_pool(name="dram", bufs=1,
                                                  space="DRAM"))

            # ---- internal DRAM ------------------------------------------
            erl0_dram = dram.tile([WIN, cfg.NW0 * H0], F16)
            h_stage_a = dram.tile([cfg.R0S, HD0], F32)
            h_stage_b = dram.tile([cfg.R0S, HD0], F32)
            z1el_shard = dram.tile([cfg.R0S, RW1 + 1], F16)
            z1el_small = dram.tile([cfg.NTAB, RW1 + 1], F16,
                                   addr_space="Shared")
            z1el_tab = dram.tile([cfg.NTAB + P, P], F16)
            o_stage_a = dram.tile([cfg.R1S, D1], F32)
            o_stage_b = dram.tile([cfg.R1S, D1], F32)

            # ---- constants ----------------------------------------------
            ident32 = const.tile([P, P], F32)
            make_identity(nc, ident32[:])
            ident16 = const.tile([P, P], F16)
            nc.vector.tensor_copy(out=ident16[:], in_=ident32[:])
            W0_sb = const.tile([F_in, HD0], F32)
            nc.sync.dma_start(out=W0_sb[:], in_=W0[:])
            Aler0_sb = const.tile([HD0, 2 * H0], F32)
            nc.sync.dma_start(out=Aler0_sb[:], in_=Aler0[:])
            b0_sb = const.tile([H0, D0], F32)
            nc.sync.dma_start(out=b0_sb[:], in_=b0r[:])
            W1_sb = const.tile([D0, HD1], F32)
            nc.sync.dma_start(out=W1_sb[:], in_=W1[:])
            Aler1_sb = const.tile([HD1, 2 * H1], F32)
            nc.sync.dma_start(out=Aler1_sb[:], in_=Aler1[:])
            b1_sb = const.tile([1, D1], F32)
            nc.sync.dma_start(out=b1_sb[:], in_=b1r[:])
            ones4_sb = const.tile([H0, 1], F32)
            nc.sync.dma_start(out=ones4_sb[:], in_=ones4[:])
            twos_sb = const.tile([1, P], F32)
            nc.sync.dma_start(out=twos_sb[:], in_=twos[:])

            pt = psm.tile([HD0, F_in], F32, tag="ps_m")
            nc.tensor.transpose(out=pt[:], in_=W0_sb[:],
                                identity=ident32[:F_in, :F_in])
            W0T_sb = const.tile([HD0, F_in], F32)
            nc.scalar.copy(out=W0T_sb[:], in_=pt[:])
            pe = psm.tile([F_in, 2 * H0], F32, tag="ps_m")
            nc.tensor.matmul(out=pe[:], lhsT=W0T_sb[:], rhs=Aler0_sb[:],
                             start=True, stop=True)
            W0el = const.tile([F_in, RW0], F16)
            nc.vector.tensor_copy(out=W0el[:, :HD0], in_=W0_sb[:])
            nc.vector.tensor_copy(out=W0el[:, HD0:], in_=pe[:, :H0])
            W0r = const.tile([F_in, H0], F16)
            nc.vector.tensor_copy(out=W0r[:], in_=pe[:, H0:])

            pt1 = psm.tile([HD1, D0], F32, tag="ps_m")
            nc.tensor.transpose(out=pt1[:], in_=W1_sb[:],
                                identity=ident32[:D0, :D0])
            W1T_sb = const.tile([HD1, D0], F32)
            nc.scalar.copy(out=W1T_sb[:], in_=pt1[:])
            pe1 = psm.tile([D0, 2 * H1], F32, tag="ps_m")
            nc.tensor.matmul(out=pe1[:], lhsT=W1T_sb[:], rhs=Aler1_sb[:],
                             start=True, stop=True)
            W1e = const.tile([D0, RW1 + 1], F16)
            nc.vector.tensor_copy(out=W1e[:, :HD1], in_=W1_sb[:])
            nc.vector.tensor_copy(out=W1e[:, HD1:], in_=pe1[:])

            ps_s = psm.tile([1, D0], F32, tag="ps_m")
            nc.tensor.matmul(out=ps_s[:], lhsT=ones4_sb[:], rhs=b0_sb[:],
                             start=True, stop=True)
            sb0_sb = const.tile([1, D0], F32)
            nc.scalar.copy(out=sb0_sb[:], in_=ps_s[:])
            ps_mb = psm.tile([P, D0], F32, tag="ps_m")
            nc.tensor.matmul(out=ps_mb[:], lhsT=twos_sb[:], rhs=sb0_sb[:],
                             start=True, stop=True)
            mb2_sb = const.tile([P, D0], F32)
            nc.scalar.copy(out=mb2_sb[:], in_=ps_mb[:])
            ps_b1 = psm.tile([P, D1], F32, tag="ps_m")
            nc.tensor.matmul(out=ps_b1[:], lhsT=twos_sb[:], rhs=b1_sb[:],
                             start=True, stop=True)
            bias1_sb = const.tile([P, D1], F32)
            nc.scalar.copy(out=bias1_sb[:], in_=ps_b1[:])

            zrow_sb = const.tile([1, RW1 + 1], F16)
            nc.vector.memset(zrow_sb[:], 0.0)

            # ---- Phase A': er0 window table -----------------------------
            n_dt = cdiv(cfg.R0S, P)
            for t in range(n_dt):
                p = min(P, cfg.R0S - t * P)
                q = p // WIN
                ftd = sbuf.tile([F_in, P], F16, tag="ftd")
                nc.sync.dma_start(out=ftd[:, :p],
                                  in_=featTdst[:, t * P:t * P + p])
                pse = psm.tile([P, H0], F32, tag="ps_m")
                nc.tensor.matmul(out=pse[:p], lhsT=ftd[:, :p], rhs=W0r[:],
                                 start=True, stop=True)
                st = sbuf.tile([P, H0], F16, tag="erst")
                nc.vector.tensor_copy(out=st[:p], in_=pse[:p])
                for qq in range(q):
                    nc.sync.dma_start(
                        out=erl0_dram[:, (4 * t + qq) * H0:
                                      (4 * t + qq + 1) * H0],
                        in_=st[qq * WIN:(qq + 1) * WIN, :])
            erl0 = const.tile([WIN, cfg.NW0 * H0], F16)
            nc.sync.dma_start(out=erl0[:], in_=erl0_dram[:])

            # ---- edge aggregation ---------------------------------------
            psw_cur = [None]

            def edge_phase(name, layer, stage_dram, erl1=None):
                ed = edge_in[name]
                ch = ed["ch"]
                h_, rw = (H0, RW0) if layer == 0 else (H1, RW1)
                hd = h_ * (D0 if layer == 0 else D1)
                nw = cfg.NW0 if layer == 0 else cfg.NW1
                stg = {"t": None}

                def flush_stage(w_hi):
                    w_lo = (w_hi // SB) * SB
                    k = w_hi - w_lo + 1
                    nc.sync.dma_start(
                        out=stage_dram[w_lo * WIN:(w_hi + 1) * WIN, :]
                        .rearrange("(j d) f -> d j f", d=WIN),
                        in_=stg["t"][:, :k * hd].rearrange(
                            "d (j f) -> d j f", f=hd))
                    stg["t"] = None

                gstep = G if layer == 0 else G1
                for g0 in range(0, ch, gstep):
                    gsz = min(gstep, ch - g0)
                    ni = gsz * P
                    cm = big.tile([P, G * WIN], F16, tag="cm")
                    nc.sync.dma_start(
                        out=cm[:, :gsz * WIN],
                        in_=ed["cm"][:, g0 * WIN:(g0 + gsz) * WIN])
                    zx = big.tile([P, G, RW0], F16, tag="zx")
                    gb = big.tile([P, G * P], F16, tag="gbig")

                    if layer == 0:
                        fe = gb
                        nc.sync.dma_start(
                            out=fe[:F_in, :ni],
                            in_=ed["fe"][:, g0 * P:(g0 + gsz) * P])
                        ct = big.tile([WIN, G * P], F16, tag="ct")
                        nc.sync.dma_start(
                            out=ct[:, :ni],
                            in_=ed["ct"][:, g0 * P:(g0 + gsz) * P])
                        for j0 in range(0, gsz, PZ):
                            bs = min(PZ, gsz - j0)
                            zps = psz.tile([P, PZ, RW0], F32, tag="ps_z")
                            for dj in range(bs):
                                j = j0 + dj
                                nc.tensor.matmul(
                                    out=zps[:, dj, :],
                                    lhsT=fe[:F_in, j * P:(j + 1) * P],
                                    rhs=W0el[:], start=True, stop=True)
                                w = (g0 + j) // CPW
                                # accumulate er onto the el columns
                                nc.tensor.matmul(
                                    out=zps[:, dj, HD0:RW0],
                                    lhsT=ct[:, j * P:(j + 1) * P],
                                    rhs=erl0[:, w * H0:(w + 1) * H0],
                                    start=False, stop=True)
                            lg = small.tile([P, PZ, H0], F32, tag="lg")
                            nc.scalar.activation(
                                out=lg[:, :bs], in_=zps[:, :bs, HD0:RW0],
                                func=AF.Prelu, alpha=NEG_SLOPE)
                            nc.scalar.activation(
                                out=zx[:, j0:j0 + bs, hd:hd + h_],
                                in_=lg[:, :bs], func=AF.Exp)
                            nc.vector.tensor_tensor(
                                out=zx[:, j0:j0 + bs, :hd].rearrange(
                                    "p j (h d) -> p j h d", h=h_),
                                in0=zps[:, :bs, :hd].rearrange(
                                    "p j (h d) -> p j h d", h=h_),
                                in1=zx[:, j0:j0 + bs, hd:hd + h_]
                                .unsqueeze(3).to_broadcast([P, bs, h_, D0]),
                                op=ALU.mult)
                    else:
                        gidx = sbuf.tile([P, G * 8], I16, tag="gidx")
                        nc.sync.dma_start(
                            out=gidx[:, :gsz * 8],
                            in_=ed["g"][:, g0 * 8:(g0 + gsz) * 8])
                        rowg = gb[:].rearrange("p (j w) -> p j w", w=P)
                        nc.gpsimd.dma_gather(
                            rowg[:, :gsz, :], z1el_tab[:], gidx[:, :gsz * 8],
                            ni, ni, P, single_packet=False)
                        ct = big.tile([WIN, G * P], F16, tag="ct")
                        nc.sync.dma_start(
                            out=ct[:, :ni],
                            in_=ed["ct"][:, g0 * P:(g0 + gsz) * P])
                        era = sbuf.tile([P, G], F32, tag="era")
                        for j0 in range(0, gsz, PZ):
                            bs = min(PZ, gsz - j0)
                            zps = psz.tile([P, PZ, RW0], F32, tag="ps_z")
                            for dj in range(bs):
                                j = j0 + dj
                                w = (g0 + j) // CPW
                                nc.tensor.matmul(
                                    out=zps[:, dj, :1],
                                    lhsT=ct[:, j * P:(j + 1) * P],
                                    rhs=erl1[:, w:w + 1],
                                    start=True, stop=True)
                            nc.vector.tensor_copy(
                                out=era[:, j0:j0 + bs],
                                in_=zps[:, :bs, 0])
                        lg = sbuf.tile([P, G, 1], F32, tag="lg1")
                        nc.vector.tensor_tensor(
                            out=lg[:, :gsz], in0=rowg[:, :gsz, hd:hd + 1],
                            in1=era[:, :gsz].unsqueeze(2), op=ALU.add)
                        nc.scalar.activation(
                            out=lg[:, :gsz], in_=lg[:, :gsz],
                            func=AF.Prelu, alpha=NEG_SLOPE)
                        nc.scalar.activation(
                            out=zx[:, :gsz, hd:hd + 1], in_=lg[:, :gsz],
                            func=AF.Exp)
                        nc.vector.tensor_tensor(
                            out=zx[:, :gsz, :hd], in0=rowg[:, :gsz, :hd],
                            in1=zx[:, :gsz, hd:hd + 1]
                            .to_broadcast([P, gsz, hd]),
                            op=ALU.mult)

                    for j in range(gsz):
                        chn = g0 + j
                        w = chn // CPW
                        first = chn % CPW == 0
                        last = chn % CPW == CPW - 1
                        if first:
                            psw_cur[0] = psw.tile(
                                [WIN, rw], F32, tag="ps_w",
                                name=f"pw_{name}_{w}")
                        nc.tensor.matmul(
                            out=psw_cur[0][:],
                            lhsT=cm[:, j * WIN:(j + 1) * WIN],
                            rhs=zx[:, j, :rw], start=first, stop=last)
                        if last:
                            pw = psw_cur[0]
                            sm = small.tile([WIN, h_], F32, tag="sm")
                            nc.vector.tensor_scalar(
                                out=sm[:], in0=pw[:, hd:hd + h_],
                                scalar1=SEG_EPS, scalar2=None, op0=ALU.max)
                            rs = small.tile([WIN, h_], F32, tag="rs")
                            nc.vector.reciprocal(out=rs[:], in_=sm[:])
                            if stg["t"] is None:
                                stg["t"] = stage.tile(
                                    [WIN, SB * hd], F32, tag="hstg",
                                    name=f"stg_{name}_{w}")
                            slot = w % SB
                            dstv = stg["t"][:, slot * hd:(slot + 1) * hd] \
                                .rearrange("d (h f) -> d h f", h=h_)
                            nc.vector.tensor_tensor(
                                out=dstv,
                                in0=pw[:, :hd].rearrange(
                                    "d (h f) -> d h f", h=h_),
                                in1=rs[:].unsqueeze(2).to_broadcast(
                                    [WIN, h_, hd // h_]),
                                op=ALU.mult)
                            if slot == SB - 1 or w == nw - 1:
                                flush_stage(w)

            # ---- Phase B: layer-0 edges ---------------------------------
            edge_phase("e0a", 0, h_stage_a)
            edge_phase("e0b", 0, h_stage_b)

            # ---- Phase C: h build + z1el shard + AllGather --------------
            n_ht = cdiv(cfg.R0S, P)
            for i in range(n_ht):
                p = min(P, cfg.R0S - i * P)
                at = sbuf.tile([P, HD0], F32, tag="ha")
                bt = sbuf.tile([P, HD0], F32, tag="hb")
                nc.sync.dma_start(out=at[:p], in_=h_stage_a[i * P:i * P + p, :])
                nc.sync.dma_start(out=bt[:p], in_=h_stage_b[i * P:i * P + p, :])
                nc.vector.tensor_add(out=at[:p], in0=at[:p], in1=bt[:p])
                hs = sbuf.tile([P, D0], F32, tag="hs")
                nc.vector.tensor_add(out=hs[:p], in0=at[:p, 0:D0],
                                     in1=at[:p, D0:2 * D0])
                for h in range(2, H0):
                    nc.vector.tensor_add(
                        out=hs[:p], in0=hs[:p],
                        in1=at[:p, h * D0:(h + 1) * D0])
                nc.vector.tensor_add(out=hs[:p], in0=hs[:p], in1=mb2_sb[:p])
                hr = sbuf.tile([P, D0], F16, tag="hr")
                nc.scalar.activation(out=hr[:p], in_=hs[:p], func=AF.Relu,
                                     scale=1.0 / H0)
                htp = psm.tile([D0, P], F16, tag="ps_m16")
                nc.tensor.transpose(out=htp[:, :p], in_=hr[:p],
                                    identity=ident16[:p, :p])
                hts = sbuf.tile([D0, P], F16, tag="hts")
                nc.vector.tensor_copy(out=hts[:, :p], in_=htp[:, :p])
                zp1 = psm.tile([P, RW1 + 1], F32, tag="ps_m")
                nc.tensor.matmul(out=zp1[:p], lhsT=hts[:, :p], rhs=W1e[:],
                                 start=True, stop=True)
                z1s = sbuf.tile([P, RW1 + 1], F16, tag="z1s")
                nc.vector.tensor_copy(out=z1s[:p], in_=zp1[:p])
                nc.sync.dma_start(out=z1el_shard[i * P:i * P + p, :],
                                  in_=z1s[:p])

            nc.gpsimd.collective_compute(
                "AllGather", ALU.bypass,
                replica_groups=[list(range(n_cores))],
                ins=[z1el_shard.opt()], outs=[z1el_small.opt()])

            # restride compact rows into 256B gather rows + zero row
            nc.sync.dma_start(out=z1el_tab[:cfg.NTAB, :RW1 + 1],
                              in_=z1el_small[:])
            nc.sync.dma_start(out=z1el_tab[cfg.ZROW:cfg.ZROW + 1, :RW1 + 1],
                              in_=zrow_sb[:])

            # er1 window values: gather dst-slot rows, extract er col into
            # erl1[s, w] (slot-partition, window-free) for the er matmuls
            e1idx = const.tile([P, NQ * 8], I16)
            nc.sync.dma_start(out=e1idx[:], in_=er1rows[:])
            e1g = const.tile([P, NQ, P], F16)
            nc.gpsimd.dma_gather(
                e1g[:], z1el_tab[:], e1idx[:], cfg.NER1, cfg.NER1, P,
                single_packet=False)
            erl1 = const.tile([WIN, NQ * CPW], F16)
            erl13 = erl1[:].rearrange("s (q c) -> s q c", c=CPW)
            for q2 in range(CPW):
                nc.sync.dma_start(
                    out=erl13[:, :, q2:q2 + 1],
                    in_=e1g[q2 * WIN:(q2 + 1) * WIN, :, RW1:RW1 + 1])

            # ---- Phase D: layer-1 edges ---------------------------------
            edge_phase("e1a", 1, o_stage_a, erl1=erl1)
            edge_phase("e1b", 1, o_stage_b, erl1=erl1)

            # ---- final combine ------------------------------------------
            n_ot = cdiv(cfg.R1S, P)
            for i in range(n_ot):
                p = min(P, cfg.R1S - i * P)
                oa = sbuf.tile([P, D1], F32, tag="oa")
                ob = sbuf.tile([P, D1], F32, tag="ob")
                nc.sync.dma_start(out=oa[:p], in_=o_stage_a[i * P:i * P + p, :])
                nc.sync.dma_start(out=ob[:p], in_=o_stage_b[i * P:i * P + p, :])
                nc.vector.tensor_add(out=oa[:p], in0=oa[:p], in1=ob[:p])
                nc.vector.tensor_add(out=oa[:p], in0=oa[:p], in1=bias1_sb[:p])
                nc.sync.dma_start(out=out_t[i * P:i * P + p, :], in_=oa[:p])

    nc.compile()
    return nc


# ----------------------------------------------------------------------------
# host driver
# ----------------------------------------------------------------------------

_CACHED = {}


def make_shared(cfg, inputs):
    return dict(
        W0=np.asarray(inputs["W0"], np.float32),
        Aler0=np.concatenate(
            [block_diag_attn(np.asarray(inputs["attn_l0"], np.float32)),
             block_diag_attn(np.asarray(inputs["attn_r0"], np.float32))],
            axis=1),
        b0r=np.asarray(inputs["b0"], np.float32).reshape(cfg.H0, cfg.D0),
        W1=np.asarray(inputs["W1"], np.float32),
        Aler1=np.concatenate(
            [block_diag_attn(np.asarray(inputs["attn_l1"], np.float32)),
             block_diag_attn(np.asarray(inputs["attn_r1"], np.float32))],
            axis=1),
        b1r=np.asarray(inputs["b1"], np.float32).reshape(1, cfg.D1),
        ones4=np.ones((cfg.H0, 1), np.float32),
        twos=np.full((1, P), 2.0, np.float32),
    )


def kernel(**inputs):
    dims = (8, 50000, 20000, 10000, 2500, 1250)
    cfg, per_core, perm1_list = prep_all(
        dims, inputs["feat"], inputs["src0a"], inputs["dst0a"],
        inputs["src0b"], inputs["dst0b"], inputs["src1a"], inputs["dst1a"],
        inputs["src1b"], inputs["dst1b"])

    key = (cfg.NW0, cfg.NW1)
    if key not in _CACHED:
        _CACHED[key] = build_program(cfg)
    nc = _CACHED[key]

    shared = make_shared(cfg, inputs)
    in_maps = []
    for c in range(cfg.n_cores):
        m = dict(shared)
        m.update(per_core[c])
        in_maps.append(m)

    res = run_bass_kernel_spmd(nc, in_maps, list(range(cfg.n_cores)))

    full = np.zeros((cfg.N2, cfg.D1), np.float32)
    for c in range(cfg.n_cores):
        o = res.results[c]["out"]
        perm1 = perm1_list[c]
        v = perm1 >= 0
        full[c * cfg.R1 + perm1[v]] = o[v]
    return full.reshape(cfg.N2, cfg.H1, cfg.D1).astype(np.float32)


# revision 33
# speedup vs baseline: 3.1364x; 1.1333x over previous
"""2-layer multi-edge-type GAT on Trainium2, 8-core SPMD (v3).

Key bottleneck on TRN2: gpsimd SWDGE descriptor generation runs at ~8-10 ns
per gathered row, so per-edge gathers cost ~1 ms/layer. This version
eliminates ALL layer-0 gathers: the edge list is known host-side, so the
host stages edge-ordered transposed feat blocks (pure data movement, no
flops) that the device reads with dense DMAs. Layer-1 sources reference the
device-computed h, so one batched int16 dma_gather per 64-chunk group
remains (the only per-edge descriptors in the kernel).

Structure per core (edges sharded by dst owner; dst nodes bin-packed into
32-slot windows jointly balanced over both edge types, <=512 edges/type per
window = exactly 4 chunks of 128):
  A) er0 window table: er = (feat @ W0)*attn_r for the permuted dst list,
     laid out [32 slots, NW0*H] for per-window matmul access.
  B) Layer-0 edges: per chunk, fp16 matmul feat_edges.T @ [W0|W0*attn_l]
     gives [z|el] in PSUM; a second small matmul cmpT.T @ er_win adds er;
     DVE: ex = exp(leakyrelu(el+er)) and rows = [ex*z | ex] fp16; one fp16
     one-hot window matmul accumulates [num|den] per window in PSUM;
     window flush normalizes (softmax shift dropped: logits O(1)).
  C) h = relu(mean_h(gat_a+gat_b)); z1el rows [z1|el1|er1] AllGathered
     compact, restrided to 256B rows for gathers.
  D) Layer-1 edges: batched dma_gather of [z1|el1] rows by src; er1 per
     edge via DVE multiply-reduce of the one-hot against window er values;
     same window aggregation; out = norm_a + norm_b + 2*b1.

Padding edges use zero feat blocks / a zero table row and all-zero one-hot
columns, so they contribute exactly nothing (ex stays finite, fp16-safe).
"""

import sys

import numpy as np

if "/opt/trn_rl_repo" not in sys.path:
    sys.path.insert(0, "/opt/trn_rl_repo")

import concourse.bacc as bacc
import concourse.bass as bass
import concourse.mybir as mybir
import concourse.tile as tile
from concourse.bass_utils import run_bass_kernel_spmd
from concourse.masks import make_identity

F32 = mybir.dt.float32
F16 = mybir.dt.float16
I16 = mybir.dt.int16
AF = mybir.ActivationFunctionType
ALU = mybir.AluOpType

P = 128
WIN = 32
CPW = 4            # chunks per window (512 edges)
CAP = WIN * 16     # 512-edge capacity per window per type
G = 64             # chunks per group (layer 0)
G1 = 28            # chunks per gather group (layer 1)
PZ = 3             # chunks per z-matmul psum batch
SB = 8             # windows per stage-flush batch
NEG_SLOPE = 0.2
SEG_EPS = 1e-9


def cdiv(a, b):
    return (a + b - 1) // b


# ----------------------------------------------------------------------------
# host-side (integer-only / data-movement-only) preprocessing
# ----------------------------------------------------------------------------

def pack_windows(dega, degb, nw):
    """Jointly bin-pack dsts into nw windows: <=32 slots, <=CAP edges of
    each type. Returns (win_of, slot_of) or None if infeasible."""
    r = len(dega)
    order = np.argsort(-(dega + degb), kind="stable")
    wca = np.zeros(nw, np.int64)
    wcb = np.zeros(nw, np.int64)
    wsl = np.zeros(nw, np.int64)
    win_of = np.full(r, -1, np.int64)
    slot_of = np.full(r, -1, np.int64)
    big = 1 << 40
    for d in order:
        da, db = dega[d], degb[d]
        cost = np.where(
            (wsl < WIN) & (wca + da <= CAP) & (wcb + db <= CAP),
            wca + wcb, big)
        w = int(np.argmin(cost))
        if cost[w] >= big:
            return None
        win_of[d] = w
        slot_of[d] = wsl[w]
        wsl[w] += 1
        wca[w] += da
        wcb[w] += db
    return win_of, slot_of


def edge_slots(src, d_loc, win_of, slot_of, nw):
    """Place edges into (chunk, partition) slots, window-major.
    Returns (srcs, cols): [CH, 128]; padding src=-1, col=-1."""
    ch = nw * CPW
    w = win_of[d_loc]
    s = slot_of[d_loc]
    order = np.lexsort((s, w))
    src_o, w_o, s_o = src[order], w[order], s[order]
    firsts = np.searchsorted(w_o, np.arange(nw), "left")
    pos = np.arange(len(w_o)) - firsts[w_o]
    assert pos.max(initial=0) < CAP
    chunk = w_o * CPW + pos // P
    part = pos % P
    srcs = np.full((ch, P), -1, np.int64)
    cols = np.full((ch, P), -1, np.int64)
    srcs[chunk, part] = src_o
    cols[chunk, part] = s_o
    return srcs, cols


def one_hots(cols):
    """cols [CH, 128] -> cm [128, CH*32] f16 and ct [32, CH*128] f16."""
    ch = cols.shape[0]
    rng32 = np.arange(WIN)
    oh = (cols[:, :, None] == rng32[None, None, :])        # [CH, 128, 32]
    cm = np.ascontiguousarray(
        oh.transpose(1, 0, 2).reshape(P, ch * WIN)).astype(np.float16)
    ct = np.ascontiguousarray(
        oh.transpose(2, 0, 1).reshape(WIN, ch * P)).astype(np.float16)
    return cm, ct


def wrap_groups(arr, group):
    """[CH, 128] int -> [128, CH*8] int16 wrapped per `group`-chunk group."""
    ch = arr.shape[0]
    out = np.zeros((128, ch * 8), np.int16)
    for g0 in range(0, ch, group):
        gsz = min(group, ch - g0)
        flat = arr[g0:g0 + gsz].reshape(-1)
        w16 = flat.reshape(-1, 16).T.astype(np.int16)
        out[:, g0 * 8:(g0 + gsz) * 8] = np.tile(w16, (8, 1))
    return out


def block_diag_attn(attn):
    h, d = attn.shape
    out = np.zeros((h * d, h), np.float32)
    for i in range(h):
        out[i * d:(i + 1) * d, i] = attn[i]
    return out


class Cfg:
    def __init__(self, nw0, nw1, n_cores=8, N0=50000, N1=20000,
                 N2=10000, F_in=64, H0=4, D0=32, H1=1, D1=32):
        self.n_cores = n_cores
        self.N0, self.N1, self.N2 = N0, N1, N2
        self.F_in, self.H0, self.D0, self.H1, self.D1 = F_in, H0, D0, H1, D1
        self.R0 = N1 // n_cores
        self.R1 = N2 // n_cores
        self.NW0, self.NW1 = nw0, nw1
        self.R0S = nw0 * WIN
        self.R1S = nw1 * WIN
        self.CH0 = nw0 * CPW
        self.CH1 = nw1 * CPW
        self.RW0 = H0 * D0 + H0   # 132
        self.RW1 = H1 * D1 + H1   # 33
        self.NTAB = n_cores * self.R0S   # z1el table rows; +1 zero row
        self.ZROW = self.NTAB
        self.NER1 = cdiv(self.R1S, P) * P


def prep_all(dims, feat, src0a, dst0a, src0b, dst0b, src1a, dst1a,
             src1b, dst1b):
    n_cores, N0, N1, N2, R0, R1 = dims
    feat = np.asarray(feat, np.float32)
    feat16 = feat.astype(np.float16)
    feat16z = np.vstack([feat16, np.zeros((1, feat.shape[1]), np.float16)])

    def split(src, dst, r):
        src, dst = np.asarray(src, np.int64), np.asarray(dst, np.int64)
        out = []
        for c in range(n_cores):
            m = (dst >= c * r) & (dst < (c + 1) * r)
            out.append((src[m], dst[m] - c * r))
        return out

    e0a, e0b = split(src0a, dst0a, R0), split(src0b, dst0b, R0)
    e1a, e1b = split(src1a, dst1a, R1), split(src1b, dst1b, R1)

    def pack_layer(ea, eb, r, nw_start):
        for nw in range(nw_start, nw_start + 6):
            packs = []
            ok = True
            for c in range(n_cores):
                dega = np.bincount(ea[c][1], minlength=r)
                degb = np.bincount(eb[c][1], minlength=r)
                pk = pack_windows(dega, degb, nw)
                if pk is None:
                    ok = False
                    break
                packs.append(pk)
            if ok:
                return nw, packs
        raise RuntimeError("window packing failed")

    nw0, packs0 = pack_layer(e0a, e0b, R0, 82)
    nw1, packs1 = pack_layer(e1a, e1b, R1, 42)
    cfg = Cfg(nw0, nw1, n_cores, N0, N1, N2)

    tabrow = np.zeros(N1, np.int64)
    for c in range(n_cores):
        win_of, slot_of = packs0[c]
        tabrow[c * R0 + np.arange(R0)] = c * cfg.R0S + win_of * WIN + slot_of

    per_core = []
    perm1_list = []
    for c in range(n_cores):
        m = {}
        win0, slot0 = packs0[c]
        win1, slot1 = packs1[c]
        for name, (src, d_loc) in (("e0a", e0a[c]), ("e0b", e0b[c])):
            srcs, cols = edge_slots(src, d_loc, win0, slot0, nw0)
            srcs[srcs < 0] = N0                    # zero feat row
            fe = feat16z[srcs.reshape(-1)].reshape(cfg.CH0, P, cfg.F_in)
            cm, ct = one_hots(cols)
            # stacked operand: rows 0:64 feat.T blocks, rows 64:96 one-hot.T
            fa = np.empty((cfg.F_in + WIN, cfg.CH0 * P), np.float16)
            fa[:cfg.F_in] = fe.transpose(2, 0, 1).reshape(
                cfg.F_in, cfg.CH0 * P)
            fa[cfg.F_in:] = ct
            m[f"{name}_fa"] = fa
            m[f"{name}_cm"] = cm
        for name, (src, d_loc) in (("e1a", e1a[c]), ("e1b", e1b[c])):
            rows = tabrow[src]
            srcs, cols = edge_slots(rows, d_loc, win1, slot1, nw1)
            srcs[srcs < 0] = cfg.ZROW
            m[f"{name}_g"] = wrap_groups(srcs, G1)
            cm, ct = one_hots(cols)
            m[f"{name}_cm"] = cm
            m[f"{name}_ct"] = ct
        # dst-permuted featT for er0 build (empty slots zero)
        ftd = np.zeros((cfg.F_in, cfg.R0S), np.float16)
        ftd[:, win0 * WIN + slot0] = feat16[c * R0:(c + 1) * R0].T
        m["featTdst"] = ftd
        # er1 window-value gather rows: slot i=(w*32+s) -> tabrow(dst)
        er1r = np.full(cfg.NER1, cfg.ZROW, np.int64)
        er1r[win1 * WIN + slot1] = tabrow[c * R1 + np.arange(R1)]
        m["er1rows"] = wrap_groups(er1r.reshape(-1, P), cdiv(cfg.NER1, P))
        per_core.append(m)
        perm1 = np.full(cfg.R1S, -1, np.int64)
        perm1[win1 * WIN + slot1] = np.arange(R1)
        perm1_list.append(perm1)
    return cfg, per_core, perm1_list


# ----------------------------------------------------------------------------
# device program
# ----------------------------------------------------------------------------

def build_program(cfg):
    n_cores = cfg.n_cores
    F_in, H0, D0, H1, D1 = cfg.F_in, cfg.H0, cfg.D0, cfg.H1, cfg.D1
    HD0, HD1 = H0 * D0, H1 * D1
    RW0, RW1 = cfg.RW0, cfg.RW1
    NQ = cfg.NER1 // P

    nc = bacc.Bacc("TRN2", target_bir_lowering=False, debug=False,
                   num_devices=n_cores)

    W0 = nc.dram_tensor("W0", [F_in, HD0], F32, kind="ExternalInput")
    Aler0 = nc.dram_tensor("Aler0", [HD0, 2 * H0], F32, kind="ExternalInput")
    b0r = nc.dram_tensor("b0r", [H0, D0], F32, kind="ExternalInput")
    W1 = nc.dram_tensor("W1", [D0, HD1], F32, kind="ExternalInput")
    Aler1 = nc.dram_tensor("Aler1", [HD1, 2 * H1], F32, kind="ExternalInput")
    b1r = nc.dram_tensor("b1r", [1, D1], F32, kind="ExternalInput")
    ones4 = nc.dram_tensor("ones4", [H0, 1], F32, kind="ExternalInput")
    twos = nc.dram_tensor("twos", [1, P], F32, kind="ExternalInput")
    featTdst = nc.dram_tensor("featTdst", [F_in, cfg.R0S], F16,
                              kind="ExternalInput")
    er1rows = nc.dram_tensor("er1rows", [P, NQ * 8], I16,
                             kind="ExternalInput")

    edge_in = {}
    for name in ("e0a", "e0b"):
        edge_in[name] = dict(
            fa=nc.dram_tensor(f"{name}_fa", [F_in + WIN, cfg.CH0 * P], F16,
                              kind="ExternalInput"),
            cm=nc.dram_tensor(f"{name}_cm", [P, cfg.CH0 * WIN], F16,
                              kind="ExternalInput"),
            ch=cfg.CH0,
        )
    for name in ("e1a", "e1b"):
        edge_in[name] = dict(
            g=nc.dram_tensor(f"{name}_g", [P, cfg.CH1 * 8], I16,
                             kind="ExternalInput"),
            cm=nc.dram_tensor(f"{name}_cm", [P, cfg.CH1 * WIN], F16,
                              kind="ExternalInput"),
            ct=nc.dram_tensor(f"{name}_ct", [WIN, cfg.CH1 * P], F16,
                              kind="ExternalInput"),
            ch=cfg.CH1,
        )

    out_t = nc.dram_tensor("out", [cfg.R1S, D1], F32, kind="ExternalOutput")

    with tile.TileContext(nc) as tc:
        from contextlib import ExitStack
        with ExitStack() as ctx:
            const = ctx.enter_context(tc.tile_pool(name="const", bufs=1))
            sbuf = ctx.enter_context(tc.tile_pool(name="sbuf", bufs=3))
            big = ctx.enter_context(tc.tile_pool(name="big", bufs=2))
            stage = ctx.enter_context(tc.tile_pool(name="stage", bufs=2))
            small = ctx.enter_context(tc.tile_pool(name="small", bufs=3))
            psm = ctx.enter_context(tc.tile_pool(name="psm", bufs=1,
                                                 space="PSUM"))
            psz = ctx.enter_context(tc.tile_pool(name="psz", bufs=3,
                                                 space="PSUM"))
            psw = ctx.enter_context(tc.tile_pool(name="psw", bufs=3,
                                                 space="PSUM"))
            dram = ctx.enter_context(tc.tile_pool(name="dram", bufs=1,
                                                  space="DRAM"))

            # ---- internal DRAM ------------------------------------------
            h_stage_a = dram.tile([cfg.R0S, HD0], F32)
            h_stage_b = dram.tile([cfg.R0S, HD0], F32)
            z1el_shard = dram.tile([cfg.R0S, RW1 + 1], F16)
            z1el_small = dram.tile([cfg.NTAB, RW1 + 1], F16,
                                   addr_space="Shared")
            z1el_tab = dram.tile([cfg.NTAB + P, P], F16)
            o_stage_a = dram.tile([cfg.R1S, D1], F32)
            o_stage_b = dram.tile([cfg.R1S, D1], F32)

            # ---- constants ----------------------------------------------
            ident32 = const.tile([P, P], F32)
            make_identity(nc, ident32[:])
            ident16 = const.tile([P, P], F16)
            nc.vector.tensor_copy(out=ident16[:], in_=ident32[:])
            W0_sb = const.tile([F_in, HD0], F32)
            nc.sync.dma_start(out=W0_sb[:], in_=W0[:])
            Aler0_sb = const.tile([HD0, 2 * H0], F32)
            nc.sync.dma_start(out=Aler0_sb[:], in_=Aler0[:])
            b0_sb = const.tile([H0, D0], F32)
            nc.sync.dma_start(out=b0_sb[:], in_=b0r[:])
            W1_sb = const.tile([D0, HD1], F32)
            nc.sync.dma_start(out=W1_sb[:], in_=W1[:])
            Aler1_sb = const.tile([HD1, 2 * H1], F32)
            nc.sync.dma_start(out=Aler1_sb[:], in_=Aler1[:])
            b1_sb = const.tile([1, D1], F32)
            nc.sync.dma_start(out=b1_sb[:], in_=b1r[:])
            ones4_sb = const.tile([H0, 1], F32)
            nc.sync.dma_start(out=ones4_sb[:], in_=ones4[:])
            twos_sb = const.tile([1, P], F32)
            nc.sync.dma_start(out=twos_sb[:], in_=twos[:])

            pt = psm.tile([HD0, F_in], F32, tag="ps_m")
            nc.tensor.transpose(out=pt[:], in_=W0_sb[:],
                                identity=ident32[:F_in, :F_in])
            W0T_sb = const.tile([HD0, F_in], F32)
            nc.scalar.copy(out=W0T_sb[:], in_=pt[:])
            pe = psm.tile([F_in, 2 * H0], F32, tag="ps_m")
            nc.tensor.matmul(out=pe[:], lhsT=W0T_sb[:], rhs=Aler0_sb[:],
                             start=True, stop=True)
            W0el = const.tile([F_in, RW0], F16)
            nc.vector.tensor_copy(out=W0el[:, :HD0], in_=W0_sb[:])
            nc.vector.tensor_copy(out=W0el[:, HD0:], in_=pe[:, :H0])
            W0r = const.tile([F_in, H0], F16)
            nc.vector.tensor_copy(out=W0r[:], in_=pe[:, H0:])

            pt1 = psm.tile([HD1, D0], F32, tag="ps_m")
            nc.tensor.transpose(out=pt1[:], in_=W1_sb[:],
                                identity=ident32[:D0, :D0])
            W1T_sb = const.tile([HD1, D0], F32)
            nc.scalar.copy(out=W1T_sb[:], in_=pt1[:])
            pe1 = psm.tile([D0, 2 * H1], F32, tag="ps_m")
            nc.tensor.matmul(out=pe1[:], lhsT=W1T_sb[:], rhs=Aler1_sb[:],
                             start=True, stop=True)
            W1e = const.tile([D0, RW1 + 1], F16)
            nc.vector.tensor_copy(out=W1e[:, :HD1], in_=W1_sb[:])
            nc.vector.tensor_copy(out=W1e[:, HD1:], in_=pe1[:])

            ps_s = psm.tile([1, D0], F32, tag="ps_m")
            nc.tensor.matmul(out=ps_s[:], lhsT=ones4_sb[:], rhs=b0_sb[:],
                             start=True, stop=True)
            sb0_sb = const.tile([1, D0], F32)
            nc.scalar.copy(out=sb0_sb[:], in_=ps_s[:])
            ps_mb = psm.tile([P, D0], F32, tag="ps_m")
            nc.tensor.matmul(out=ps_mb[:], lhsT=twos_sb[:], rhs=sb0_sb[:],
                             start=True, stop=True)
            mb2_sb = const.tile([P, D0], F32)
            nc.scalar.copy(out=mb2_sb[:], in_=ps_mb[:])
            ps_b1 = psm.tile([P, D1], F32, tag="ps_m")
            nc.tensor.matmul(out=ps_b1[:], lhsT=twos_sb[:], rhs=b1_sb[:],
                             start=True, stop=True)
            bias1_sb = const.tile([P, D1], F32)
            nc.scalar.copy(out=bias1_sb[:], in_=ps_b1[:])

            zrow_sb = const.tile([1, RW1 + 1], F16)
            nc.vector.memset(zrow_sb[:], 0.0)

            # ---- Phase A': er0 window table (straight into SBUF) --------
            # erl0 lives on partitions 64:96 so the per-window copy into
            # the stacked-matmul rhs stays partition-aligned.
            erl0 = const.tile([F_in + WIN, cfg.NW0 * H0], F16)
            ftd_sb = const.tile([F_in, cfg.R0S], F16)
            nc.sync.dma_start(out=ftd_sb[:], in_=featTdst[:])
            n_dt = cdiv(cfg.R0S, P)
            for t in range(n_dt):
                p = min(P, cfg.R0S - t * P)
                q = p // WIN
                pse = psm.tile([P, H0], F32, tag="ps_m")
                nc.tensor.matmul(out=pse[:p],
                                 lhsT=ftd_sb[:, t * P:t * P + p],
                                 rhs=W0r[:], start=True, stop=True)
                st = sbuf.tile([P, H0], F16, tag="erst")
                nc.vector.tensor_copy(out=st[:p], in_=pse[:p])
                for qq in range(q):
                    nc.sync.dma_start(
                        out=erl0[F_in:, (4 * t + qq) * H0:
                                 (4 * t + qq + 1) * H0],
                        in_=st[qq * WIN:(qq + 1) * WIN, :])

            # three rotating stacked-rhs tiles: [W0el ; 0 | er_w]
            rhs_bufs = []
            for rb in range(3):
                rbuf = const.tile([F_in + WIN, RW0], F16,
                                  name=f"rhsw{rb}")
                nc.vector.tensor_copy(out=rbuf[:F_in, :], in_=W0el[:])
                nc.vector.memset(rbuf[F_in:, :], 0.0)
                rhs_bufs.append(rbuf)

            # ---- edge aggregation ---------------------------------------
            psw_cur = [None]

            def edge_phase(name, layer, stage_dram, erl1=None):
                ed = edge_in[name]
                ch = ed["ch"]
                h_, rw = (H0, RW0) if layer == 0 else (H1, RW1)
                hd = h_ * (D0 if layer == 0 else D1)
                nw = cfg.NW0 if layer == 0 else cfg.NW1
                stg = {"t": None}

                def flush_stage(w_hi):
                    w_lo = (w_hi // SB) * SB
                    k = w_hi - w_lo + 1
                    nc.sync.dma_start(
                        out=stage_dram[w_lo * WIN:(w_hi + 1) * WIN, :]
                        .rearrange("(j d) f -> d j f", d=WIN),
                        in_=stg["t"][:, :k * hd].rearrange(
                            "d (j f) -> d j f", f=hd))
                    stg["t"] = None

                gstep = G if layer == 0 else G1
                for g0 in range(0, ch, gstep):
                    gsz = min(gstep, ch - g0)
                    ni = gsz * P
                    cm = big.tile([P, G * WIN], F16, tag="cm")
                    nc.sync.dma_start(
                        out=cm[:, :gsz * WIN],
                        in_=ed["cm"][:, g0 * WIN:(g0 + gsz) * WIN])
                    zx = big.tile([P, G, RW0], F16, tag="zx")

                    if layer == 0:
                        fa = big.tile([F_in + WIN, G * P], F16, tag="fa")
                        nc.sync.dma_start(
                            out=fa[:, :ni],
                            in_=ed["fa"][:, g0 * P:(g0 + gsz) * P])
                        for j0 in range(0, gsz, PZ):
                            bs = min(PZ, gsz - j0)
                            zps = psz.tile([P, PZ, RW0], F32, tag="ps_z")
                            for dj in range(bs):
                                j = j0 + dj
                                w = (g0 + j) // CPW
                                rbuf = rhs_bufs[w % 3]
                                if (g0 + j) % CPW == 0:
                                    nc.vector.tensor_copy(
                                        out=rbuf[F_in:, HD0:RW0],
                                        in_=erl0[F_in:,
                                                 w * H0:(w + 1) * H0])
                                nc.tensor.matmul(
                                    out=zps[:, dj, :],
                                    lhsT=fa[:, j * P:(j + 1) * P],
                                    rhs=rbuf[:], start=True, stop=True)
                            lg = small.tile([P, PZ, H0], F32, tag="lg")
                            nc.scalar.activation(
                                out=lg[:, :bs], in_=zps[:, :bs, HD0:RW0],
                                func=AF.Prelu, alpha=NEG_SLOPE)
                            nc.scalar.activation(
                                out=zx[:, j0:j0 + bs, hd:hd + h_],
                                in_=lg[:, :bs], func=AF.Exp)
                            nc.vector.tensor_tensor(
                                out=zx[:, j0:j0 + bs, :hd].rearrange(
                                    "p j (h d) -> p j h d", h=h_),
                                in0=zps[:, :bs, :hd].rearrange(
                                    "p j (h d) -> p j h d", h=h_),
                                in1=zx[:, j0:j0 + bs, hd:hd + h_]
                                .unsqueeze(3).to_broadcast([P, bs, h_, D0]),
                                op=ALU.mult)
                    else:
                        gidx = sbuf.tile([P, G * 8], I16, tag="gidx")
                        nc.sync.dma_start(
                            out=gidx[:, :gsz * 8],
                            in_=ed["g"][:, g0 * 8:(g0 + gsz) * 8])
                        rowg = big.tile([P, G1, P], F16, tag="rowg")
                        nc.gpsimd.dma_gather(
                            rowg[:, :gsz, :], z1el_tab[:], gidx[:, :gsz * 8],
                            ni, ni, P, single_packet=False)
                        ct = big.tile([WIN, G1 * P], F16, tag="ct")
                        nc.sync.dma_start(
                            out=ct[:, :ni],
                            in_=ed["ct"][:, g0 * P:(g0 + gsz) * P])
                        era = sbuf.tile([P, G], F32, tag="era")
                        for j0 in range(0, gsz, PZ):
                            bs = min(PZ, gsz - j0)
                            zps = psz.tile([P, PZ, RW0], F32, tag="ps_z")
                            for dj in range(bs):
                                j = j0 + dj
                                w = (g0 + j) // CPW
                                nc.tensor.matmul(
                                    out=zps[:, dj, :1],
                                    lhsT=ct[:, j * P:(j + 1) * P],
                                    rhs=erl1[:, w:w + 1],
                                    start=True, stop=True)
                            nc.vector.tensor_copy(
                                out=era[:, j0:j0 + bs],
                                in_=zps[:, :bs, 0])
                        lg = sbuf.tile([P, G, 1], F32, tag="lg1")
                        nc.vector.tensor_tensor(
                            out=lg[:, :gsz], in0=rowg[:, :gsz, hd:hd + 1],
                            in1=era[:, :gsz].unsqueeze(2), op=ALU.add)
                        nc.scalar.activation(
                            out=lg[:, :gsz], in_=lg[:, :gsz],
                            func=AF.Prelu, alpha=NEG_SLOPE)
                        nc.scalar.activation(
                            out=zx[:, :gsz, hd:hd + 1], in_=lg[:, :gsz],
                            func=AF.Exp)
                        nc.vector.tensor_tensor(
                            out=zx[:, :gsz, :hd], in0=rowg[:, :gsz, :hd],
                            in1=zx[:, :gsz, hd:hd + 1]
                            .to_broadcast([P, gsz, hd]),
                            op=ALU.mult)

                    for j in range(gsz):
                        chn = g0 + j
                        w = chn // CPW
                        first = chn % CPW == 0
                        last = chn % CPW == CPW - 1
                        if first:
                            psw_cur[0] = psw.tile(
                                [WIN, rw], F32, tag="ps_w",
                                name=f"pw_{name}_{w}")
                        nc.tensor.matmul(
                            out=psw_cur[0][:],
                            lhsT=cm[:, j * WIN:(j + 1) * WIN],
                            rhs=zx[:, j, :rw], start=first, stop=last)
                        if last:
                            pw = psw_cur[0]
                            sm = small.tile([WIN, h_], F32, tag="sm")
                            nc.vector.tensor_scalar(
                                out=sm[:], in0=pw[:, hd:hd + h_],
                                scalar1=SEG_EPS, scalar2=None, op0=ALU.max)
                            rs = small.tile([WIN, h_], F32, tag="rs")
                            nc.vector.reciprocal(out=rs[:], in_=sm[:])
                            if stg["t"] is None:
                                stg["t"] = stage.tile(
                                    [WIN, SB * hd], F32, tag="hstg",
                                    name=f"stg_{name}_{w}")
                            slot = w % SB
                            dstv = stg["t"][:, slot * hd:(slot + 1) * hd] \
                                .rearrange("d (h f) -> d h f", h=h_)
                            nc.vector.tensor_tensor(
                                out=dstv,
                                in0=pw[:, :hd].rearrange(
                                    "d (h f) -> d h f", h=h_),
                                in1=rs[:].unsqueeze(2).to_broadcast(
                                    [WIN, h_, hd // h_]),
                                op=ALU.mult)
                            if slot == SB - 1 or w == nw - 1:
                                flush_stage(w)

            # ---- Phase B: layer-0 edges ---------------------------------
            edge_phase("e0a", 0, h_stage_a)
            edge_phase("e0b", 0, h_stage_b)

            # ---- Phase C: h build + z1el shard + AllGather --------------
            n_ht = cdiv(cfg.R0S, P)
            for i in range(n_ht):
                p = min(P, cfg.R0S - i * P)
                at = sbuf.tile([P, HD0], F32, tag="ha")
                bt = sbuf.tile([P, HD0], F32, tag="hb")
                nc.sync.dma_start(out=at[:p], in_=h_stage_a[i * P:i * P + p, :])
                nc.sync.dma_start(out=bt[:p], in_=h_stage_b[i * P:i * P + p, :])
                nc.vector.tensor_add(out=at[:p], in0=at[:p], in1=bt[:p])
                hs = sbuf.tile([P, D0], F32, tag="hs")
                nc.vector.tensor_add(out=hs[:p], in0=at[:p, 0:D0],
                                     in1=at[:p, D0:2 * D0])
                for h in range(2, H0):
                    nc.vector.tensor_add(
                        out=hs[:p], in0=hs[:p],
                        in1=at[:p, h * D0:(h + 1) * D0])
                nc.vector.tensor_add(out=hs[:p], in0=hs[:p], in1=mb2_sb[:p])
                hr = sbuf.tile([P, D0], F16, tag="hr")
                nc.scalar.activation(out=hr[:p], in_=hs[:p], func=AF.Relu,
                                     scale=1.0 / H0)
                htp = psm.tile([D0, P], F16, tag="ps_m16")
                nc.tensor.transpose(out=htp[:, :p], in_=hr[:p],
                                    identity=ident16[:p, :p])
                hts = sbuf.tile([D0, P], F16, tag="hts")
                nc.vector.tensor_copy(out=hts[:, :p], in_=htp[:, :p])
                zp1 = psm.tile([P, RW1 + 1], F32, tag="ps_m")
                nc.tensor.matmul(out=zp1[:p], lhsT=hts[:, :p], rhs=W1e[:],
                                 start=True, stop=True)
                z1s = sbuf.tile([P, RW1 + 1], F16, tag="z1s")
                nc.vector.tensor_copy(out=z1s[:p], in_=zp1[:p])
                nc.sync.dma_start(out=z1el_shard[i * P:i * P + p, :],
                                  in_=z1s[:p])

            nc.gpsimd.collective_compute(
                "AllGather", ALU.bypass,
                replica_groups=[list(range(n_cores))],
                ins=[z1el_shard.opt()], outs=[z1el_small.opt()])

            # restride compact rows into 256B gather rows + zero row
            nc.sync.dma_start(out=z1el_tab[:cfg.NTAB, :RW1 + 1],
                              in_=z1el_small[:])
            nc.sync.dma_start(out=z1el_tab[cfg.ZROW:cfg.ZROW + 1, :RW1 + 1],
                              in_=zrow_sb[:])

            # er1 window values: gather dst-slot rows, extract er col into
            # erl1[s, w] (slot-partition, window-free) for the er matmuls
            e1idx = const.tile([P, NQ * 8], I16)
            nc.sync.dma_start(out=e1idx[:], in_=er1rows[:])
            e1g = const.tile([P, NQ, P], F16)
            nc.gpsimd.dma_gather(
                e1g[:], z1el_tab[:], e1idx[:], cfg.NER1, cfg.NER1, P,
                single_packet=False)
            erl1 = const.tile([WIN, NQ * CPW], F16)
            erl13 = erl1[:].rearrange("s (q c) -> s q c", c=CPW)
            for q2 in range(CPW):
                nc.sync.dma_start(
                    out=erl13[:, :, q2:q2 + 1],
                    in_=e1g[q2 * WIN:(q2 + 1) * WIN, :, RW1:RW1 + 1])

            # ---- Phase D: layer-1 edges ---------------------------------
            edge_phase("e1a", 1, o_stage_a, erl1=erl1)
            edge_phase("e1b", 1, o_stage_b, erl1=erl1)

            # ---- final combine ------------------------------------------
            n_ot = cdiv(cfg.R1S, P)
            for i in range(n_ot):
                p = min(P, cfg.R1S - i * P)
                oa = sbuf.tile([P, D1], F32, tag="oa")
                ob = sbuf.tile([P, D1], F32, tag="ob")
                nc.sync.dma_start(out=oa[:p], in_=o_stage_a[i * P:i * P + p, :])
                nc.sync.dma_start(out=ob[:p], in_=o_stage_b[i * P:i * P + p, :])
                nc.vector.tensor_add(out=oa[:p], in0=oa[:p], in1=ob[:p])
                nc.vector.tensor_add(out=oa[:p], in0=oa[:p], in1=bias1_sb[:p])
                nc.sync.dma_start(out=out_t[i * P:i * P + p, :], in_=oa[:p])

    nc.compile()
    return nc


# ----------------------------------------------------------------------------
# host driver
# ----------------------------------------------------------------------------

_CACHED = {}


def make_shared(cfg, inputs):
    return dict(
        W0=np.asarray(inputs["W0"], np.float32),
        Aler0=np.concatenate(
            [block_diag_attn(np.asarray(inputs["attn_l0"], np.float32)),
             block_diag_attn(np.asarray(inputs["attn_r0"], np.float32))],
            axis=1),
        b0r=np.asarray(inputs["b0"], np.float32).reshape(cfg.H0, cfg.D0),
        W1=np.asarray(inputs["W1"], np.float32),
        Aler1=np.concatenate(
            [block_diag_attn(np.asarray(inputs["attn_l1"], np.float32)),
             block_diag_attn(np.asarray(inputs["attn_r1"], np.float32))],
            axis=1),
        b1r=np.asarray(inputs["b1"], np.float32).reshape(1, cfg.D1),
        ones4=np.ones((cfg.H0, 1), np.float32),
        twos=np.full((1, P), 2.0, np.float32),
    )


def kernel(**inputs):
    dims = (8, 50000, 20000, 10000, 2500, 1250)
    cfg, per_core, perm1_list = prep_all(
        dims, inputs["feat"], inputs["src0a"], inputs["dst0a"],
        inputs["src0b"], inputs["dst0b"], inputs["src1a"], inputs["dst1a"],
        inputs["src1b"], inputs["dst1b"])

    key = (cfg.NW0, cfg.NW1)
    if key not in _CACHED:
        _CACHED[key] = build_program(cfg)
    nc = _CACHED[key]

    shared = make_shared(cfg, inputs)
    in_maps = []
    for c in range(cfg.n_cores):
        m = dict(shared)
        m.update(per_core[c])
        in_maps.append(m)

    res = run_bass_kernel_spmd(nc, in_maps, list(range(cfg.n_cores)))

    full = np.zeros((cfg.N2, cfg.D1), np.float32)
    for c in range(cfg.n_cores):
        o = res.results[c]["out"]
        perm1 = perm1_list[c]
        v = perm1 >= 0
        full[c * cfg.R1 + perm1[v]] = o[v]
    return full.reshape(cfg.N2, cfg.H1, cfg.D1).astype(np.float32)
